# revision 18
# baseline (speedup 1.0000x reference)
"""Trainium2 Bass kernel for nn_Autoencoder_65223373357102 (FLAME-style autoencoder).

Strategy (v6):
  Phase 1 (8-way tensor parallel): encoder GEMM sharded along K, W packed to
  the 411 *used* latent columns. The fp32 GEMM is decomposed into three bf16
  passes (x_hi*W_hi + x_hi*W_lo + x_lo*W_hi, fp32 PSUM accumulation): bf16
  products are exact in fp32, so the latent error is ~4e-6 relative - inside
  the ~1e-5 budget set by the z-clamped projection - while the PE runs 1
  cycle/row instead of fp32's 4. x (hi+lo) is SBUF-resident; W streams in 21
  pre-tiled contiguous chunks on two DMA queues, deep-buffered so the NRT
  start barrier overlaps prefetch. Bias (scaled 1/8) and a constant 1/8 lane
  (col 411) are folded into the PSUM accumulation; the AllReduce of [64,412]
  then yields latent + an exact 1.0 in col 411 that phase 2 uses as the
  template coefficient.
  Phase 2 (8-way vertex parallel): each core computes only its 448 of the 3500
  face verts plus 72 synthetic columns (68 landmarks, l/r eye means, face
  centre, vmean) via an fp32 [64,400+]@[400+,1560] GEMM from host-gathered
  basis columns. Everything the reference does to the eye vertex slices is
  dead code w.r.t. the output (only the eye means and gaze rotations survive).
  shape_p is transposed on the PE (identity matmul). The latent AllReduce is
  an AllGather + local tree-reduce (fewer RDH steps), preceded by a warm-up
  AllGather that absorbs the collective firmware wakeup (~11us -> ~1us
  trigger delay). The gaze solve uses the closed form for unit gaze vectors
  (det = 1 - (lg.rg)^2) instead of a general 3x3 Cramer chain. Per-core
  output [64,3,975] is stitched to the full [64,7079,3] on the host.
"""
import sys
import types

sys.path.insert(0, "/opt/trn_rl_repo")

import numpy as np
import ml_dtypes

BF = ml_dtypes.bfloat16


def _ensure_ntff_hook():
    """Provide antenv.axon_hooks + install the ctypes NTFF profile hook so
    run_bass_kernel_spmd(trace=True) can pull a neuron-profile under axon."""
    name = "antenv.axon_hooks"
    if name not in sys.modules:
        mod = types.ModuleType(name)
        mod._HOOK = None

        def set_axon_ntff_profile_hook(hook):
            mod._HOOK = hook

        def get_axon_ntff_profile_hook():
            return mod._HOOK

        mod.set_axon_ntff_profile_hook = set_axon_ntff_profile_hook
        mod.get_axon_ntff_profile_hook = get_axon_ntff_profile_hook
        sys.modules[name] = mod
        try:
            import antenv

            antenv.axon_hooks = mod
        except ImportError:
            pass
    mod = sys.modules[name]
    if mod.get_axon_ntff_profile_hook() is None:
        try:
            from trn_agent_boot.trn_boot import _ntff_profile_via_ctypes

            hook = _ntff_profile_via_ctypes("/opt/axon/libaxon_pjrt.so")
            if hook is not None:
                mod.set_axon_ntff_profile_hook(hook)
        except Exception:
            pass


_ensure_ntff_hook()

from concourse import bass, mybir, tile
from concourse.bass_utils import run_bass_kernel_spmd

F32 = mybir.dt.float32
BF16 = mybir.dt.bfloat16
ALU = mybir.AluOpType
ACTF = mybir.ActivationFunctionType
AX = mybir.AxisListType

B = 64
V = 5023
VM = 3500
LAT = 556
DIN = 3 * 224 * 224  # 150528
NCORES = 8
KSH = DIN // NCORES  # 18816
KT = KSH // 128  # 147 k-tiles
TPC = 7  # k-tiles per W chunk
NCH = KT // TPC  # 21 chunks
NCOLS = 411  # packed latent cols: 0:400 + 545:556
NOUT = 2 * VM + 68 + 11  # 7079
SL = 448  # verts per core (last core: 364 real + pad)
PL = SL + 68 + 4  # per-plane block: slice, fl, lme, rme, fc, vmean = 520
N2 = 3 * PL  # 1560
GAZE_DIR = -1.0
HALF_PI = 1.5707963267948966
# packed pose col offsets (orig 545:556 -> packed 400:411)
P_ROT, P_T, P_SC, P_LR, P_RR = 400, 403, 406, 407, 409


class Geo:
    """Helper for tiny per-batch scalar ops on [rows,1] tiles."""

    _uid = [0]

    def __init__(self, nc, pool, rows=B, eng=None):
        self.nc = nc
        self.pool = pool
        self.rows = rows
        self.eng = eng if eng is not None else nc.vector

    def t(self, cols=1):
        Geo._uid[0] += 1
        return self.pool.tile([self.rows, cols], F32, name=f"g{Geo._uid[0]}_{cols}")

    def mul(self, a, b):
        o = self.t()
        self.eng.tensor_tensor(out=o, in0=a, in1=b, op=ALU.mult)
        return o

    def add(self, a, b):
        o = self.t()
        self.eng.tensor_tensor(out=o, in0=a, in1=b, op=ALU.add)
        return o

    def sub(self, a, b):
        o = self.t()
        self.eng.tensor_tensor(out=o, in0=a, in1=b, op=ALU.subtract)
        return o

    def mac(self, a, s, acc):
        """(a * s) + acc, s is a [rows,1] AP scalar."""
        o = self.t()
        self.eng.scalar_tensor_tensor(
            out=o, in0=a, scalar=s, in1=acc, op0=ALU.mult, op1=ALU.add
        )
        return o

    def dot3(self, ax, ay, az, bx, by, bz):
        o = self.mul(ax, bx)
        o = self.mac(ay, by, o)
        o = self.mac(az, bz, o)
        return o

    def cross3(self, ax, ay, az, bx, by, bz):
        cx = self.sub(self.mul(ay, bz), self.mul(az, by))
        cy = self.sub(self.mul(az, bx), self.mul(ax, bz))
        cz = self.sub(self.mul(ax, by), self.mul(ay, bx))
        return cx, cy, cz


def axis_angle_R(nc, g, aa3, pfx, halfpi):
    R_ = g.rows
    """aa3: [rows,3] axis-angle tile -> (D, MI, PL3) [rows,3] tiles with
    R[0,0],R[1,1],R[2,2] = D[:,0..2]
    R[0,1],R[1,2],R[2,0] = MI[:,0..2]  (m - s terms)
    R[1,0],R[2,1],R[0,2] = PL3[:,0..2] (m + s terms)
    """
    pool = g.pool
    eng = g.eng
    sq = pool.tile([R_, 3], F32, name=pfx + "aaR_sq")
    eng.tensor_tensor(out=sq, in0=aa3, in1=aa3, op=ALU.mult)
    th2a = g.t()
    eng.tensor_tensor(out=th2a, in0=sq[:, 0:1], in1=sq[:, 1:2], op=ALU.add)
    th2 = g.t()
    eng.tensor_tensor(out=th2, in0=th2a, in1=sq[:, 2:3], op=ALU.add)
    theta = g.t()
    nc.scalar.activation(out=theta, in_=th2, func=ACTF.Sqrt)
    thm = g.t()
    eng.tensor_scalar_max(out=thm, in0=theta, scalar1=1e-8)
    rth = g.t()
    nc.vector.reciprocal(out=rth, in_=thm)
    axis3 = pool.tile([R_, 3], F32, name=pfx + "aaR_axis")
    eng.tensor_scalar_mul(out=axis3, in0=aa3, scalar1=rth)
    s = g.t()
    nc.scalar.activation(out=s, in_=theta, func=ACTF.Sin)
    c = g.t()
    nc.scalar.activation(out=c, in_=theta, func=ACTF.Sin, bias=halfpi)
    omc = g.t()
    eng.tensor_scalar(
        out=omc, in0=c, scalar1=-1.0, scalar2=1.0, op0=ALU.mult, op1=ALU.add
    )
    asq = pool.tile([R_, 3], F32, name=pfx + "aaR_asq")
    eng.tensor_tensor(out=asq, in0=axis3, in1=axis3, op=ALU.mult)
    dmul = pool.tile([R_, 3], F32, name=pfx + "aaR_dmul")
    eng.tensor_scalar_mul(out=dmul, in0=asq, scalar1=omc)
    D = pool.tile([R_, 3], F32, name=pfx + "aaR_D")
    eng.tensor_scalar(out=D, in0=dmul, scalar1=c, op0=ALU.add, scalar2=None)
    # m3 = (ax*ay, ay*az, az*ax) * omc ; s3 = (s*az, s*ax, s*ay)
    r1 = pool.tile([R_, 3], F32, name=pfx + "aaR_r1")
    eng.tensor_copy(out=r1[:, 0:2], in_=axis3[:, 1:3])
    eng.tensor_copy(out=r1[:, 2:3], in_=axis3[:, 0:1])
    m3 = pool.tile([R_, 3], F32, name=pfx + "aaR_m3")
    eng.tensor_tensor(out=m3, in0=axis3, in1=r1, op=ALU.mult)
    eng.tensor_scalar_mul(out=m3, in0=m3, scalar1=omc)
    sa = pool.tile([R_, 3], F32, name=pfx + "aaR_sa")
    eng.tensor_scalar_mul(out=sa, in0=axis3, scalar1=s)
    sr = pool.tile([R_, 3], F32, name=pfx + "aaR_sr")
    eng.tensor_copy(out=sr[:, 0:1], in_=sa[:, 2:3])
    eng.tensor_copy(out=sr[:, 1:3], in_=sa[:, 0:2])
    MI = pool.tile([R_, 3], F32, name=pfx + "aaR_MI")
    eng.tensor_tensor(out=MI, in0=m3, in1=sr, op=ALU.subtract)
    PL3 = pool.tile([R_, 3], F32, name=pfx + "aaR_PL")
    eng.tensor_tensor(out=PL3, in0=m3, in1=sr, op=ALU.add)
    return D, MI, PL3


def r_at(D, MI, PL3, l, i):
    """R[l,i] as a [rows,1] AP from the (D, MI, PL3) triple."""
    if l == i:
        return D[:, l:l + 1]
    m = {(0, 1): MI[:, 0:1], (1, 2): MI[:, 1:2], (2, 0): MI[:, 2:3],
         (1, 0): PL3[:, 0:1], (2, 1): PL3[:, 1:2], (0, 2): PL3[:, 2:3]}
    return m[(l, i)]


_ENG_ATTR = {
    "SP": "sync", "Pool": "gpsimd", "PE": "tensor",
    "DVE": "vector", "Activation": "scalar",
}


def _legalize_waits(nc):
    """This walrus accepts only one sync-wait slot per instruction; move extra
    waits onto same-engine NoOps inserted right before the instruction."""
    import concourse.mybir as _mybir

    def make_nop(engine):
        eng = getattr(nc, _ENG_ATTR[engine.name])
        bi = eng.nop(nofuse=True)
        mi = bi.ins
        for bb in nc.main_func.blocks:
            if bb.instructions and bb.instructions[-1].name == mi.name:
                bb.instructions.pop()
                break
        mi.engine = engine
        return mi

    for bb in nc.main_func.blocks:
        snapshot = list(bb.instructions)
        newlist = []
        changed = False
        for inst in snapshot:
            si = inst.sync_info
            waits = list(si.on_wait) if (si and si.on_wait) else []
            if (
                len(waits) > 1
                and not inst.name.startswith("barrier")
                and inst.engine is not None
                and getattr(inst.engine, "name", None) in _ENG_ATTR
            ):
                for w in waits[:-1]:
                    nop = make_nop(inst.engine)
                    nop.sync_info = _mybir.SyncInfo(on_wait=[w], on_update=[])
                    newlist.append(nop)
                inst.sync_info = _mybir.SyncInfo(
                    on_wait=[waits[-1]], on_update=list(si.on_update)
                )
                changed = True
            newlist.append(inst)
        if changed:
            bb.instructions[:] = newlist
    return nc


XPARTS = [25, 25, 25, 25, 25, 22]  # k-tile split of the resident x shard


def build_graph():
    nc = bass.Bass(target_bir_lowering=False)

    x_p = nc.declare_dram_parameter("xw", [128, KT, 2, B], BF16, isOutput=False)
    w_p = nc.declare_dram_parameter(
        "wch", [NCH, 128, TPC, 2, NCOLS], BF16, isOutput=False
    )
    b_p = nc.declare_dram_parameter("bvec", [1, NCOLS + 1], F32, isOutput=False)
    bas_p = nc.declare_dram_parameter("basis", [128, 4, N2], F32, isOutput=False)
    cam_p = nc.declare_dram_parameter("cam", [B, 12], F32, isOutput=False)
    eye_p = nc.declare_dram_parameter("eye", [B, B], F32, isOutput=False)
    out_p = nc.declare_dram_parameter("out", [B, 3, 2 * SL + 71 + 8], F32, isOutput=True)

    ar_in = nc.dram_tensor("ar_in", [B, NCOLS + 1], F32)
    ag_out = nc.dram_tensor("ag_out", [NCORES * B, NCOLS + 1], F32, addr_space="Shared")
    warm_in = nc.dram_tensor("warm_in", [1, 4], F32)
    warm_out = nc.dram_tensor("warm_out", [NCORES, 4], F32, addr_space="Shared")

    with tile.TileContext(nc) as tc:
        with (
            tc.tile_pool(name="consts", bufs=1) as consts,
            tc.tile_pool(name="xres", bufs=1) as xres,
            tc.tile_pool(name="latp", bufs=1) as latp,
            tc.tile_pool(name="geop", bufs=1) as geop,
            tc.tile_pool(name="planep", bufs=1) as planep,
            tc.tile_pool(name="dum", bufs=1, space="PSUM") as dum,
        ):
            # ---- const / prefetch loads ----
            eye_sb = consts.tile([B, B], F32)
            nc.scalar.dma_start(out=eye_sb, in_=eye_p[:, :])
            cam = consts.tile([B, 12], F32)
            nc.scalar.dma_start(out=cam, in_=cam_p[:, :])
            b_sb = consts.tile([1, NCOLS + 1], F32)
            nc.scalar.dma_start(out=b_sb, in_=b_p[:, :])
            ones1 = consts.tile([1, B], F32)
            nc.vector.memset(ones1, 1.0)
            halfpi = consts.tile([128, 1], F32)
            nc.vector.memset(halfpi, HALF_PI)
            lat = latp.tile([B, 416], F32)
            nc.vector.memset(lat, 0.0)

            # warm up the collective firmware path so the real gather below
            # does not pay first-op wakeup costs
            nc.gpsimd.collective_compute(
                "AllGather",
                ALU.bypass,
                replica_groups=[list(range(NCORES))],
                ins=[warm_in.ap().opt()],
                outs=[warm_out.ap().opt()],
            )

            # resident x shard (hi+lo), split so PE can start after part 0
            xts = []
            off = 0
            for pi, n in enumerate(XPARTS):
                xt = xres.tile([128, n, 2, B], BF16, name=f"xt{pi}")
                nc.gpsimd.dma_start(out=xt, in_=x_p[:, off:off + n, :, :])
                xts.append((off, n, xt))
                off += n
            # phase-2 basis block (prefetch; lands during phase 1)
            basis_sb = planep.tile([128, 4, N2], F32)
            nc.gpsimd.dma_start(out=basis_sb, in_=bas_p[:, :, :])

            def xap(k, hl):
                for off, n, xt in xts:
                    if k < off + n:
                        return xt[:, k - off, hl, :]
                raise IndexError(k)

            d1 = dum.tile([1, 1], F32)

            # ---------------- Phase 1: encoder GEMM (hi/lo bf16) ----------------
            with (
                tc.tile_pool(name="wts", bufs=9) as wts,
                tc.tile_pool(name="encp", bufs=1, space="PSUM") as encp,
            ):
                pe = encp.tile([B, NCOLS + 1], F32)
                nc.tensor.matmul(
                    d1, lhsT=xts[0][2][:, 0, 0, 0:1], rhs=xts[0][2][:, 0, 0, 0:1],
                    start=True, stop=True, skip_group_check=True,
                )
                for ci in range(NCH):
                    w_c = wts.tile([128, TPC, 2, NCOLS], BF16)
                    eng = (nc.sync, nc.scalar, nc.gpsimd)[ci % 3]
                    eng.dma_start(out=w_c, in_=w_p[ci])
                    for t in range(TPC):
                        k = ci * TPC + t
                        nc.tensor.matmul(
                            pe[:, 0:NCOLS], lhsT=xap(k, 0), rhs=w_c[:, t, 0, :],
                            start=(k == 0), stop=False,
                        )
                        nc.tensor.matmul(
                            pe[:, 0:NCOLS], lhsT=xap(k, 0), rhs=w_c[:, t, 1, :],
                            start=False, stop=False,
                        )
                        nc.tensor.matmul(
                            pe[:, 0:NCOLS], lhsT=xap(k, 1), rhs=w_c[:, t, 0, :],
                            start=False, stop=False,
                        )
                # bias (scaled 1/8) + constant 1/8 lane in col 411, fp32
                nc.tensor.matmul(
                    d1, lhsT=b_sb[0:1, 0:1], rhs=b_sb[0:1, 0:1],
                    start=True, stop=True, skip_group_check=True,
                )
                nc.tensor.matmul(
                    pe, lhsT=ones1, rhs=b_sb, start=False, stop=True,
                )
                lat1 = latp.tile([B, NCOLS + 1], F32)
                nc.vector.tensor_copy(out=lat1, in_=pe)
                nc.sync.dma_start(out=ar_in[:, :], in_=lat1)

            nc.gpsimd.collective_compute(
                "AllGather",
                ALU.bypass,
                replica_groups=[list(range(NCORES))],
                ins=[ar_in.ap().opt()],
                outs=[ag_out.ap().opt()],
            )
            # keep the PE pipeline warm through the collective window so the
            # blendshape GEMM runs at full clock
            with tc.tile_pool(name="warmp", bufs=1, space="PSUM") as warmp:
                wdum = warmp.tile([B, 512], F32)
                for _ in range(40):
                    nc.tensor.matmul(
                        wdum, lhsT=basis_sb[0:B, 0, 0:B],
                        rhs=basis_sb[0:B, 0, 0:512],
                        start=True, stop=True, skip_group_check=True,
                    )

            parts = latp.tile([B, NCORES, NCOLS + 1], F32)
            ag_v = ag_out.ap().rearrange("(c b) n -> b c n", b=B)
            nc.sync.dma_start(out=parts[:, 0:4, :], in_=ag_v[:, 0:4, :])
            nc.scalar.dma_start(out=parts[:, 4:8, :], in_=ag_v[:, 4:8, :])
            nc.vector.tensor_tensor(
                out=parts[:, 0:4, :], in0=parts[:, 0:4, :], in1=parts[:, 4:8, :],
                op=ALU.add,
            )
            nc.vector.tensor_tensor(
                out=parts[:, 0:2, :], in0=parts[:, 0:2, :], in1=parts[:, 2:4, :],
                op=ALU.add,
            )
            nc.vector.tensor_tensor(
                out=lat[:, 0:NCOLS + 1], in0=parts[:, 0, :], in1=parts[:, 1, :],
                op=ALU.add,
            )

            # ---------------- Phase 1.5: transpose shape params on PE ----------
            with tc.tile_pool(name="trps", bufs=1, space="PSUM") as trps:
                trp = trps.tile([128, 4, B], F32)
                nc.tensor.matmul(
                    d1, lhsT=eye_sb[0:1, 0:1], rhs=eye_sb[0:1, 0:1],
                    start=True, stop=True, skip_group_check=True,
                )
                for kt in range(3):
                    nc.tensor.matmul(
                        trp[:, kt, :], lhsT=lat[:, kt * 128:(kt + 1) * 128],
                        rhs=eye_sb, is_transpose=True,
                        start=True, stop=True, skip_group_check=True,
                    )
                nc.tensor.matmul(
                    trp[0:32, 3, :], lhsT=lat[:, 384:416],
                    rhs=eye_sb, is_transpose=True,
                    start=True, stop=True, skip_group_check=True,
                )
                spT = latp.tile([128, 4, B], F32)
                nc.scalar.copy(out=spT, in_=trp)

            # ---------------- Phase 2: blendshape GEMM (V-sharded) -------------
            vpre = planep.tile([B, N2], F32)
            NSPL2 = [(0, 512), (512, 512), (1024, 512), (1536, N2 - 1536)]
            with tc.tile_pool(name="p2ps", bufs=1, space="PSUM") as p2ps:
                pvs = [
                    p2ps.tile([B, n], F32, name=f"pv{j}", tag=f"pv{j}")
                    for j, (_, n) in enumerate(NSPL2)
                ]
                nc.tensor.matmul(
                    d1, lhsT=basis_sb[0:1, 0, 0:1], rhs=basis_sb[0:1, 0, 0:1],
                    start=True, stop=True, skip_group_check=True,
                )
                for j, (n0, n) in enumerate(NSPL2):
                    for kt in range(4):
                        rows = 128 if kt < 3 else 32
                        nc.tensor.matmul(
                            pvs[j],
                            lhsT=spT[0:rows, kt, :],
                            rhs=basis_sb[0:rows, kt, n0:n0 + n],
                            start=(kt == 0),
                            stop=(kt == 3),
                        )

                # eyeball rotation inputs (copies allowed on GpSimd)
                aa2 = geop.tile([128, 3], F32)
                nc.gpsimd.memset(aa2, 0.0)
                nc.gpsimd.tensor_copy(out=aa2[0:B, 0:2], in_=lat[:, P_LR:P_LR + 2])
                nc.sync.dma_start(out=aa2[B:128, 0:2], in_=lat[:, P_RR:P_RR + 2])

                # face rotation (DVE, overlaps the GEMM)
                g = Geo(nc, geop)
                fD, fMI, fPL = axis_angle_R(
                    nc, g, lat[:, P_ROT:P_ROT + 3], "f_", halfpi[:B, :]
                )
                fs = g.t()
                nc.vector.tensor_scalar_add(
                    out=fs, in0=lat[:, P_SC:P_SC + 1], scalar1=1.0
                )
                RsD = geop.tile([B, 3], F32)
                RsMI = geop.tile([B, 3], F32)
                RsPL = geop.tile([B, 3], F32)
                nc.vector.tensor_scalar_mul(out=RsD, in0=fD, scalar1=fs)
                nc.vector.tensor_scalar_mul(out=RsMI, in0=fMI, scalar1=fs)
                nc.vector.tensor_scalar_mul(out=RsPL, in0=fPL, scalar1=fs)

                def rs(l, i):
                    return r_at(RsD, RsMI, RsPL, l, i)

                # eyeball rotations: DVE work that needs only lat; fills the
                # DVE idle while the PE runs the blendshape GEMM
                g2 = Geo(nc, geop, rows=128)
                eD, eMI, ePL = axis_angle_R(nc, g2, aa2, "e_", halfpi)
                gz = geop.tile([128, 3], F32)
                nc.vector.tensor_scalar_mul(
                    out=gz[:, 0:1], in0=r_at(eD, eMI, ePL, 2, 0), scalar1=GAZE_DIR
                )
                nc.vector.tensor_scalar_mul(
                    out=gz[:, 1:2], in0=r_at(eD, eMI, ePL, 2, 1), scalar1=GAZE_DIR
                )
                nc.vector.tensor_scalar_mul(
                    out=gz[:, 2:3], in0=r_at(eD, eMI, ePL, 2, 2), scalar1=GAZE_DIR
                )
                rg64 = geop.tile([B, 3], F32)
                nc.sync.dma_start(out=rg64, in_=gz[B:128, :])
                lg = [gz[0:B, i:i + 1] for i in range(3)]
                rg = [rg64[:, i:i + 1] for i in range(3)]

                for j, (n0, n) in enumerate([NSPL2[3], NSPL2[1], NSPL2[2], NSPL2[0]]):
                    nc.scalar.copy(out=vpre[:, n0:n0 + n], in_=pvs[NSPL2.index((n0, n))])

            # offsets: off_i = face_t_i - sum_l vms_l*Rs[l,i]
            off3 = geop.tile([B, 3], F32)
            for i in range(3):
                t = g.mul(vpre[:, 519:520], rs(0, i))
                t = g.mac(vpre[:, 520 + 519:520 + 520], rs(1, i), t)
                t = g.mac(vpre[:, 1040 + 519:1040 + 520], rs(2, i), t)
                nc.vector.tensor_tensor(
                    out=off3[:, i:i + 1], in0=lat[:, P_T + i:P_T + i + 1], in1=t,
                    op=ALU.subtract,
                )

            # rotate + translate all plane blocks
            rt = planep.tile([B, 3, PL], F32)
            for i in range(3):
                nc.vector.tensor_scalar(
                    out=rt[:, i, :], in0=vpre[:, 0:PL],
                    scalar1=rs(0, i), scalar2=off3[:, i:i + 1],
                    op0=ALU.mult, op1=ALU.add,
                )
                for l in (1, 2):
                    nc.vector.scalar_tensor_tensor(
                        out=rt[:, i, :], in0=vpre[:, l * PL:(l + 1) * PL],
                        scalar=rs(l, i),
                        in1=rt[:, i, :],
                        op0=ALU.mult, op1=ALU.add,
                    )

            lc = [rt[:, i, SL + 68:SL + 69] for i in range(3)]
            rc = [rt[:, i, SL + 69:SL + 70] for i in range(3)]

            # projection of this core's vert slice (DVE)
            with tc.tile_pool(name="imgp", bufs=1) as imgp:
                img = imgp.tile([B, 3, SL], F32)
                for i in (2, 0, 1):  # z first (feeds the clamp chain)
                    nc.vector.tensor_scalar(
                        out=img[:, i, :], in0=rt[:, 0, 0:SL],
                        scalar1=cam[:, 4 * i:4 * i + 1],
                        scalar2=cam[:, 4 * i + 3:4 * i + 4],
                        op0=ALU.mult, op1=ALU.add,
                    )
                    for l in (1, 2):
                        nc.vector.scalar_tensor_tensor(
                            out=img[:, i, :], in0=rt[:, l, 0:SL],
                            scalar=cam[:, 4 * i + l:4 * i + l + 1], in1=img[:, i, :],
                            op0=ALU.mult, op1=ALU.add,
                        )
                az_ = imgp.tile([B, SL], F32)
                nc.scalar.activation(out=az_, in_=img[:, 2, :], func=ACTF.Abs)
                nc.vector.tensor_scalar_max(out=az_, in0=az_, scalar1=1e-3)
                sg = imgp.tile([B, SL], F32)
                nc.vector.tensor_scalar(
                    out=sg, in0=img[:, 2, :], scalar1=0.0, scalar2=None, op0=ALU.is_ge
                )
                nc.vector.tensor_scalar(
                    out=sg, in0=sg, scalar1=2.0, scalar2=1.0,
                    op0=ALU.mult, op1=ALU.subtract,
                )
                nc.vector.tensor_tensor(out=sg, in0=sg, in1=az_, op=ALU.mult)
                nc.vector.reciprocal(out=az_, in_=sg)
                nc.vector.tensor_tensor(
                    out=img[:, 0, :], in0=img[:, 0, :], in1=az_, op=ALU.mult
                )
                nc.vector.tensor_tensor(
                    out=img[:, 1, :], in0=img[:, 1, :], in1=az_, op=ALU.mult
                )

                # vert + img outputs can ship while the tail is computed
                nc.sync.dma_start(out=out_p[:, :, 0:SL], in_=rt[:, :, 0:SL])
                nc.scalar.dma_start(out=out_p[:, :, SL:2 * SL], in_=img)
                nc.sync.dma_start(
                    out=out_p[:, :, 2 * SL:2 * SL + 71], in_=rt[:, :, SL:SL + 71]
                )

                # tail block ge[:, i, j]: gp_l gp_r gp_mid far_l far_r lg rg dist
                ge = geop.tile([B, 3, 8], F32)
                for i in range(3):
                    # independent pieces off the DVE critical chain
                    nc.vector.scalar_tensor_tensor(
                        out=ge[:, i, 3:4], in0=lg[i], scalar=1000.0,
                        in1=lc[i], op0=ALU.mult, op1=ALU.add,
                    )
                    nc.vector.scalar_tensor_tensor(
                        out=ge[:, i, 4:5], in0=rg[i], scalar=1000.0,
                        in1=rc[i], op0=ALU.mult, op1=ALU.add,
                    )
                    nc.gpsimd.tensor_copy(out=ge[:, i, 5:6], in_=lg[i])
                    nc.gpsimd.tensor_copy(out=ge[:, i, 6:7], in_=rg[i])

                # gaze intersection: with |lg|=|rg|=1 the Cramer solve
                # collapses to c=lg.rg, det=1-c^2, sol0=(d.lg - c d.rg)/det,
                # sol1=(c d.lg - d.rg)/det  (triple-product expansion of
                # A=[lg, -rg, rg x lg])
                d3 = geop.tile([B, 3, 1], F32)
                nc.vector.tensor_tensor(
                    out=d3, in0=rt[:, :, SL + 69:SL + 70],
                    in1=rt[:, :, SL + 68:SL + 69], op=ALU.subtract,
                )
                lg3 = gz[0:B, :]
                rg3 = rg64
                dv = d3[:, :, 0]
                cw = g.t(3)
                nc.vector.tensor_tensor(out=cw, in0=lg3, in1=rg3, op=ALU.mult)
                c_ = g.t()
                nc.vector.tensor_reduce(out=c_, in_=cw, axis=AX.X, op=ALU.add)
                pw = g.t(3)
                nc.vector.tensor_tensor(out=pw, in0=dv, in1=lg3, op=ALU.mult)
                p_ = g.t()
                nc.vector.tensor_reduce(out=p_, in_=pw, axis=AX.X, op=ALU.add)
                qw = g.t(3)
                nc.vector.tensor_tensor(out=qw, in0=dv, in1=rg3, op=ALU.mult)
                q_ = g.t()
                nc.vector.tensor_reduce(out=q_, in_=qw, axis=AX.X, op=ALU.add)
                csq = g.mul(c_, c_)
                det = g.t()
                nc.vector.tensor_scalar(
                    out=det, in0=csq, scalar1=-1.0, scalar2=1.0,
                    op0=ALU.mult, op1=ALU.add,
                )
                rdet = g.t()
                nc.vector.reciprocal(out=rdet, in_=det)
                sol0 = g.mul(g.sub(p_, g.mul(c_, q_)), rdet)
                sol1 = g.mul(g.sub(g.mul(c_, p_), q_), rdet)

                gpl = geop.tile([B, 3], F32)
                gpr = geop.tile([B, 3], F32)
                for i in range(3):
                    nc.vector.scalar_tensor_tensor(
                        out=gpl[:, i:i + 1], in0=lg[i], scalar=sol0,
                        in1=lc[i], op0=ALU.mult, op1=ALU.add,
                    )
                    nc.vector.scalar_tensor_tensor(
                        out=gpr[:, i:i + 1], in0=rg[i], scalar=sol1,
                        in1=rc[i], op0=ALU.mult, op1=ALU.add,
                    )
                    nc.vector.tensor_copy(out=ge[:, i, 0:1], in_=gpl[:, i:i + 1])
                    nc.vector.tensor_copy(out=ge[:, i, 1:2], in_=gpr[:, i:i + 1])
                    o = g.add(gpl[:, i:i + 1], gpr[:, i:i + 1])
                    nc.vector.tensor_scalar_mul(out=ge[:, i, 2:3], in0=o, scalar1=0.5)
                dff = geop.tile([B, 3], F32)
                nc.vector.tensor_tensor(out=dff, in0=gpl, in1=gpr, op=ALU.subtract)
                nc.vector.tensor_tensor(out=dff, in0=dff, in1=dff, op=ALU.mult)
                d2 = g.t()
                nc.vector.tensor_reduce(out=d2, in_=dff, axis=AX.X, op=ALU.add)
                dist = g.t()
                nc.scalar.activation(out=dist, in_=d2, func=ACTF.Sqrt)
                for i in range(3):
                    nc.scalar.copy(out=ge[:, i, 7:8], in_=dist)

                nc.scalar.dma_start(out=out_p[:, :, 2 * SL + 71:2 * SL + 79], in_=ge)
    _legalize_waits(nc)
    return nc


def _prep(inputs):
    f32 = np.float32
    x = np.ascontiguousarray(inputs["x"].reshape(B, DIN), dtype=f32)
    W = np.asarray(inputs["enc_W"], dtype=f32)
    Wp = np.concatenate([W[:, :400], W[:, 545:556]], axis=1)  # [DIN, 411]
    enc_b = np.asarray(inputs["enc_b"], dtype=f32)
    bp = np.concatenate([enc_b[:400], enc_b[545:556]])
    bvec = np.concatenate(
        [bp / NCORES, np.array([1.0 / NCORES], f32)]
    ).reshape(1, NCOLS + 1).astype(f32)
    tmpl = np.asarray(inputs["v_template"], dtype=f32)  # [V, 3]
    basis = np.asarray(inputs["shape_basis"], dtype=f32)  # [400, V, 3]
    cam = np.ascontiguousarray(
        np.asarray(inputs["camera_parameters"], dtype=f32).reshape(B, 12)
    )
    lm = np.asarray(inputs["landmarks"])
    mlm = np.asarray(inputs["masked_landmarks"])
    fmask = np.asarray(inputs["face_mask"])
    lmask = np.asarray(inputs["left_eyeball_mask"])
    rmask = np.asarray(inputs["right_eyeball_mask"])
    fl_idx = fmask[mlm]  # verts behind the 68 output landmarks
    idx4 = lm[np.array([19, 22, 25, 28])]
    idx2 = lm[np.array([14, 18])]

    # synthetic extra columns [400, 72, 3] / [72, 3]
    ex_b = np.concatenate([
        basis[:, fl_idx, :],
        basis[:, lmask, :].mean(axis=1, keepdims=True),
        basis[:, rmask, :].mean(axis=1, keepdims=True),
        (basis[:, idx4, :].mean(axis=1, keepdims=True)
         + basis[:, idx2, :].mean(axis=1, keepdims=True)) / 2.0,
        basis.mean(axis=1, keepdims=True),
    ], axis=1)
    ex_t = np.concatenate([
        tmpl[fl_idx],
        tmpl[lmask].mean(axis=0, keepdims=True),
        tmpl[rmask].mean(axis=0, keepdims=True),
        (tmpl[idx4].mean(axis=0, keepdims=True)
         + tmpl[idx2].mean(axis=0, keepdims=True)) / 2.0,
        tmpl.mean(axis=0, keepdims=True),
    ], axis=0)

    eye = np.eye(B, dtype=f32)
    in_maps = []
    for c in range(NCORES):
        k0 = c * KSH
        xs = x[:, k0:k0 + KSH].T  # [KSH, B] f32
        xh = xs.astype(BF)
        xl = (xs - xh.astype(f32)).astype(BF)
        xw = np.ascontiguousarray(
            np.stack([
                xh.reshape(KT, 128, B).transpose(1, 0, 2),
                xl.reshape(KT, 128, B).transpose(1, 0, 2),
            ], axis=2)
        )  # [128, KT, 2, B] bf16
        ws = Wp[k0:k0 + KSH]  # [KSH, 411] f32
        wh = ws.astype(BF)
        wl = (ws - wh.astype(f32)).astype(BF)
        wch = np.ascontiguousarray(
            np.stack([
                wh.reshape(NCH, TPC, 128, NCOLS).transpose(0, 2, 1, 3),
                wl.reshape(NCH, TPC, 128, NCOLS).transpose(0, 2, 1, 3),
            ], axis=3)
        )  # [NCH, 128, TPC, 2, 411] bf16

        lo = c * SL
        verts = fmask[lo:min(lo + SL, VM)]
        nsl = len(verts)
        blk = np.zeros((400, N2), f32)
        trow = np.zeros(N2, f32)
        for l in range(3):
            blk[:, l * PL:l * PL + nsl] = basis[:, verts, l]
            blk[:, l * PL + SL:l * PL + SL + 72] = ex_b[:, :, l]
            trow[l * PL:l * PL + nsl] = tmpl[verts, l]
            trow[l * PL + SL:l * PL + SL + 72] = ex_t[:, l]
        bh = np.zeros((128, 4, N2), f32)
        for kt in range(3):
            bh[:, kt, :] = blk[kt * 128:(kt + 1) * 128]
        bh[0:16, 3, :] = blk[384:400]
        bh[27, 3, :] = trow  # coefficient = exact 1.0 from AR col 411
        in_maps.append({
            "xw": xw,
            "wch": wch,
            "bvec": bvec,
            "basis": np.ascontiguousarray(bh),
            "cam": cam,
            "eye": eye,
        })
    return in_maps


def _run(inputs, trace=False):
    in_maps = _prep(inputs)
    nc = build_graph()
    res = run_bass_kernel_spmd(
        nc, in_maps, core_ids=list(range(NCORES)), trace=trace
    )
    full = np.empty((B, 3, NOUT), np.float32)
    for c in range(NCORES):
        r = res.results[c]["out"]  # [B, 3, 975]
        lo = c * SL
        w = min(SL, VM - lo)
        full[:, :, lo:lo + w] = r[:, :, 0:w]
        full[:, :, VM + lo:VM + lo + w] = r[:, :, SL:SL + w]
    r0 = res.results[0]["out"]
    full[:, :, 2 * VM:NOUT] = r0[:, :, 2 * SL:2 * SL + 79]
    return np.ascontiguousarray(full.transpose(0, 2, 1)), res


def kernel(**inputs):
    out, _ = _run(inputs, trace=False)
    return out


# revision 19
# speedup vs baseline: 1.1440x; 1.1440x over previous
"""Trainium2 Bass kernel for nn_Autoencoder_65223373357102 (FLAME-style autoencoder).

Strategy (v6):
  Phase 1 (8-way tensor parallel): encoder GEMM sharded along K, W packed to
  the 411 *used* latent columns. The fp32 GEMM is decomposed into three bf16
  passes (x_hi*W_hi + x_hi*W_lo + x_lo*W_hi, fp32 PSUM accumulation): bf16
  products are exact in fp32, so the latent error is ~4e-6 relative - inside
  the ~1e-5 budget set by the z-clamped projection - while the PE runs 1
  cycle/row instead of fp32's 4. x (hi+lo) is SBUF-resident; W streams in 21
  pre-tiled contiguous chunks on two DMA queues, deep-buffered so the NRT
  start barrier overlaps prefetch. Bias (scaled 1/8) and a constant 1/8 lane
  (col 411) are folded into the PSUM accumulation; the AllReduce of [64,412]
  then yields latent + an exact 1.0 in col 411 that phase 2 uses as the
  template coefficient.
  Phase 2 (8-way vertex parallel): each core computes only its 448 of the 3500
  face verts plus 72 synthetic columns (68 landmarks, l/r eye means, face
  centre, vmean) via an fp32 [64,400+]@[400+,1560] GEMM from host-gathered
  basis columns. Everything the reference does to the eye vertex slices is
  dead code w.r.t. the output (only the eye means and gaze rotations survive).
  shape_p is transposed on the PE (identity matmul). The latent AllReduce is
  an AllGather + local tree-reduce (fewer RDH steps), preceded by a warm-up
  AllGather that absorbs the collective firmware wakeup (~11us -> ~1us
  trigger delay). The gaze solve uses the closed form for unit gaze vectors
  (det = 1 - (lg.rg)^2) instead of a general 3x3 Cramer chain. Per-core
  output [64,3,975] is stitched to the full [64,7079,3] on the host.
"""
import sys
import types

sys.path.insert(0, "/opt/trn_rl_repo")

import numpy as np
import ml_dtypes

BF = ml_dtypes.bfloat16


def _ensure_ntff_hook():
    """Provide antenv.axon_hooks + install the ctypes NTFF profile hook so
    run_bass_kernel_spmd(trace=True) can pull a neuron-profile under axon."""
    name = "antenv.axon_hooks"
    if name not in sys.modules:
        mod = types.ModuleType(name)
        mod._HOOK = None

        def set_axon_ntff_profile_hook(hook):
            mod._HOOK = hook

        def get_axon_ntff_profile_hook():
            return mod._HOOK

        mod.set_axon_ntff_profile_hook = set_axon_ntff_profile_hook
        mod.get_axon_ntff_profile_hook = get_axon_ntff_profile_hook
        sys.modules[name] = mod
        try:
            import antenv

            antenv.axon_hooks = mod
        except ImportError:
            pass
    mod = sys.modules[name]
    if mod.get_axon_ntff_profile_hook() is None:
        try:
            from trn_agent_boot.trn_boot import _ntff_profile_via_ctypes

            hook = _ntff_profile_via_ctypes("/opt/axon/libaxon_pjrt.so")
            if hook is not None:
                mod.set_axon_ntff_profile_hook(hook)
        except Exception:
            pass


_ensure_ntff_hook()

from concourse import bass, mybir, tile
from concourse.bass_utils import run_bass_kernel_spmd

F32 = mybir.dt.float32
BF16 = mybir.dt.bfloat16
ALU = mybir.AluOpType
ACTF = mybir.ActivationFunctionType
AX = mybir.AxisListType

B = 64
V = 5023
VM = 3500
LAT = 556
DIN = 3 * 224 * 224  # 150528
NCORES = 8
KSH = DIN // NCORES  # 18816
KT = KSH // 128  # 147 k-tiles
TPC = 7  # k-tiles per W chunk
NCH = KT // TPC  # 21 chunks
NCOLS = 411  # packed latent cols: 0:400 + 545:556
NOUT = 2 * VM + 68 + 11  # 7079
SL = 448  # verts per core (last core: 364 real + pad)
PL = SL + 68 + 4  # per-plane block: slice, fl, lme, rme, fc, vmean = 520
N2 = 3 * PL  # 1560
GAZE_DIR = -1.0
HALF_PI = 1.5707963267948966
# packed pose col offsets (orig 545:556 -> packed 400:411)
P_ROT, P_T, P_SC, P_LR, P_RR = 400, 403, 406, 407, 409


class Geo:
    """Helper for tiny per-batch scalar ops on [rows,1] tiles."""

    _uid = [0]

    def __init__(self, nc, pool, rows=B, eng=None):
        self.nc = nc
        self.pool = pool
        self.rows = rows
        self.eng = eng if eng is not None else nc.vector

    def t(self, cols=1):
        Geo._uid[0] += 1
        return self.pool.tile([self.rows, cols], F32, name=f"g{Geo._uid[0]}_{cols}")

    def mul(self, a, b):
        o = self.t()
        self.eng.tensor_tensor(out=o, in0=a, in1=b, op=ALU.mult)
        return o

    def add(self, a, b):
        o = self.t()
        self.eng.tensor_tensor(out=o, in0=a, in1=b, op=ALU.add)
        return o

    def sub(self, a, b):
        o = self.t()
        self.eng.tensor_tensor(out=o, in0=a, in1=b, op=ALU.subtract)
        return o

    def mac(self, a, s, acc):
        """(a * s) + acc, s is a [rows,1] AP scalar."""
        o = self.t()
        self.eng.scalar_tensor_tensor(
            out=o, in0=a, scalar=s, in1=acc, op0=ALU.mult, op1=ALU.add
        )
        return o

    def dot3(self, ax, ay, az, bx, by, bz):
        o = self.mul(ax, bx)
        o = self.mac(ay, by, o)
        o = self.mac(az, bz, o)
        return o

    def cross3(self, ax, ay, az, bx, by, bz):
        cx = self.sub(self.mul(ay, bz), self.mul(az, by))
        cy = self.sub(self.mul(az, bx), self.mul(ax, bz))
        cz = self.sub(self.mul(ax, by), self.mul(ay, bx))
        return cx, cy, cz


def axis_angle_R(nc, g, aa3, pfx, halfpi):
    R_ = g.rows
    """aa3: [rows,3] axis-angle tile -> (D, MI, PL3) [rows,3] tiles with
    R[0,0],R[1,1],R[2,2] = D[:,0..2]
    R[0,1],R[1,2],R[2,0] = MI[:,0..2]  (m - s terms)
    R[1,0],R[2,1],R[0,2] = PL3[:,0..2] (m + s terms)
    """
    pool = g.pool
    eng = g.eng
    sq = pool.tile([R_, 3], F32, name=pfx + "aaR_sq")
    eng.tensor_tensor(out=sq, in0=aa3, in1=aa3, op=ALU.mult)
    th2a = g.t()
    eng.tensor_tensor(out=th2a, in0=sq[:, 0:1], in1=sq[:, 1:2], op=ALU.add)
    th2 = g.t()
    eng.tensor_tensor(out=th2, in0=th2a, in1=sq[:, 2:3], op=ALU.add)
    theta = g.t()
    nc.scalar.activation(out=theta, in_=th2, func=ACTF.Sqrt)
    thm = g.t()
    eng.tensor_scalar_max(out=thm, in0=theta, scalar1=1e-8)
    rth = g.t()
    nc.vector.reciprocal(out=rth, in_=thm)
    axis3 = pool.tile([R_, 3], F32, name=pfx + "aaR_axis")
    eng.tensor_scalar_mul(out=axis3, in0=aa3, scalar1=rth)
    s = g.t()
    nc.scalar.activation(out=s, in_=theta, func=ACTF.Sin)
    c = g.t()
    nc.scalar.activation(out=c, in_=theta, func=ACTF.Sin, bias=halfpi)
    omc = g.t()
    eng.tensor_scalar(
        out=omc, in0=c, scalar1=-1.0, scalar2=1.0, op0=ALU.mult, op1=ALU.add
    )
    asq = pool.tile([R_, 3], F32, name=pfx + "aaR_asq")
    eng.tensor_tensor(out=asq, in0=axis3, in1=axis3, op=ALU.mult)
    dmul = pool.tile([R_, 3], F32, name=pfx + "aaR_dmul")
    eng.tensor_scalar_mul(out=dmul, in0=asq, scalar1=omc)
    D = pool.tile([R_, 3], F32, name=pfx + "aaR_D")
    eng.tensor_scalar(out=D, in0=dmul, scalar1=c, op0=ALU.add, scalar2=None)
    # m3 = (ax*ay, ay*az, az*ax) * omc ; s3 = (s*az, s*ax, s*ay)
    r1 = pool.tile([R_, 3], F32, name=pfx + "aaR_r1")
    eng.tensor_copy(out=r1[:, 0:2], in_=axis3[:, 1:3])
    eng.tensor_copy(out=r1[:, 2:3], in_=axis3[:, 0:1])
    m3 = pool.tile([R_, 3], F32, name=pfx + "aaR_m3")
    eng.tensor_tensor(out=m3, in0=axis3, in1=r1, op=ALU.mult)
    eng.tensor_scalar_mul(out=m3, in0=m3, scalar1=omc)
    sa = pool.tile([R_, 3], F32, name=pfx + "aaR_sa")
    eng.tensor_scalar_mul(out=sa, in0=axis3, scalar1=s)
    sr = pool.tile([R_, 3], F32, name=pfx + "aaR_sr")
    eng.tensor_copy(out=sr[:, 0:1], in_=sa[:, 2:3])
    eng.tensor_copy(out=sr[:, 1:3], in_=sa[:, 0:2])
    MI = pool.tile([R_, 3], F32, name=pfx + "aaR_MI")
    eng.tensor_tensor(out=MI, in0=m3, in1=sr, op=ALU.subtract)
    PL3 = pool.tile([R_, 3], F32, name=pfx + "aaR_PL")
    eng.tensor_tensor(out=PL3, in0=m3, in1=sr, op=ALU.add)
    return D, MI, PL3


def r_at(D, MI, PL3, l, i):
    """R[l,i] as a [rows,1] AP from the (D, MI, PL3) triple."""
    if l == i:
        return D[:, l:l + 1]
    m = {(0, 1): MI[:, 0:1], (1, 2): MI[:, 1:2], (2, 0): MI[:, 2:3],
         (1, 0): PL3[:, 0:1], (2, 1): PL3[:, 1:2], (0, 2): PL3[:, 2:3]}
    return m[(l, i)]


_ENG_ATTR = {
    "SP": "sync", "Pool": "gpsimd", "PE": "tensor",
    "DVE": "vector", "Activation": "scalar",
}


def _legalize_waits(nc):
    """This walrus accepts only one sync-wait slot per instruction; move extra
    waits onto same-engine NoOps inserted right before the instruction."""
    import concourse.mybir as _mybir

    def make_nop(engine):
        eng = getattr(nc, _ENG_ATTR[engine.name])
        bi = eng.nop(nofuse=True)
        mi = bi.ins
        for bb in nc.main_func.blocks:
            if bb.instructions and bb.instructions[-1].name == mi.name:
                bb.instructions.pop()
                break
        mi.engine = engine
        return mi

    for bb in nc.main_func.blocks:
        snapshot = list(bb.instructions)
        newlist = []
        changed = False
        for inst in snapshot:
            si = inst.sync_info
            waits = list(si.on_wait) if (si and si.on_wait) else []
            if (
                len(waits) > 1
                and not inst.name.startswith("barrier")
                and inst.engine is not None
                and getattr(inst.engine, "name", None) in _ENG_ATTR
            ):
                for w in waits[:-1]:
                    nop = make_nop(inst.engine)
                    nop.sync_info = _mybir.SyncInfo(on_wait=[w], on_update=[])
                    newlist.append(nop)
                inst.sync_info = _mybir.SyncInfo(
                    on_wait=[waits[-1]], on_update=list(si.on_update)
                )
                changed = True
            newlist.append(inst)
        if changed:
            bb.instructions[:] = newlist
    return nc


XPARTS = [25, 25, 25, 25, 25, 22]  # k-tile split of the resident x shard


def build_graph():
    nc = bass.Bass(target_bir_lowering=False)

    x_p = nc.declare_dram_parameter("xw", [128, KT, 2, B], BF16, isOutput=False)
    w_p = nc.declare_dram_parameter(
        "wch", [NCH, 128, TPC, 2, NCOLS], BF16, isOutput=False
    )
    b_p = nc.declare_dram_parameter("bvec", [1, NCOLS + 1], F32, isOutput=False)
    bas_p = nc.declare_dram_parameter("basis", [128, 4, N2], F32, isOutput=False)
    cam_p = nc.declare_dram_parameter("cam", [B, 12], F32, isOutput=False)
    eye_p = nc.declare_dram_parameter("eye", [B, B], F32, isOutput=False)
    out_p = nc.declare_dram_parameter("out", [B, 3, 2 * SL + 71 + 8], F32, isOutput=True)

    ar_in = nc.dram_tensor("ar_in", [B, NCOLS + 1], F32)
    ag_out = nc.dram_tensor("ag_out", [NCORES * B, NCOLS + 1], F32, addr_space="Shared")
    warm_in = nc.dram_tensor("warm_in", [1, 4], F32)
    warm_out = nc.dram_tensor("warm_out", [NCORES, 4], F32, addr_space="Shared")

    with tile.TileContext(nc) as tc:
        with (
            tc.tile_pool(name="consts", bufs=1) as consts,
            tc.tile_pool(name="xres", bufs=1) as xres,
            tc.tile_pool(name="latp", bufs=1) as latp,
            tc.tile_pool(name="geop", bufs=1) as geop,
            tc.tile_pool(name="planep", bufs=1) as planep,
            tc.tile_pool(name="dum", bufs=1, space="PSUM") as dum,
        ):
            # ---- const / prefetch loads ----
            eye_sb = consts.tile([B, B], F32)
            nc.scalar.dma_start(out=eye_sb, in_=eye_p[:, :])
            cam = consts.tile([B, 12], F32)
            nc.scalar.dma_start(out=cam, in_=cam_p[:, :])
            b_sb = consts.tile([1, NCOLS + 1], F32)
            nc.scalar.dma_start(out=b_sb, in_=b_p[:, :])
            ones1 = consts.tile([1, B], F32)
            nc.vector.memset(ones1, 1.0)
            halfpi = consts.tile([128, 1], F32)
            nc.vector.memset(halfpi, HALF_PI)
            lat = latp.tile([B, 416], F32)
            nc.vector.memset(lat, 0.0)

            # warm up the collective firmware path so the real gather below
            # does not pay first-op wakeup costs
            nc.gpsimd.collective_compute(
                "AllGather",
                ALU.bypass,
                replica_groups=[list(range(NCORES))],
                ins=[warm_in.ap().opt()],
                outs=[warm_out.ap().opt()],
            )

            # resident x shard (hi+lo), split so PE can start after part 0
            xts = []
            off = 0
            for pi, n in enumerate(XPARTS):
                xt = xres.tile([128, n, 2, B], BF16, name=f"xt{pi}")
                nc.gpsimd.dma_start(out=xt, in_=x_p[:, off:off + n, :, :])
                xts.append((off, n, xt))
                off += n
            # phase-2 basis block (prefetch; lands during phase 1)
            basis_sb = planep.tile([128, 4, N2], F32)
            nc.gpsimd.dma_start(out=basis_sb, in_=bas_p[:, :, :])

            def xap(k, hl):
                for off, n, xt in xts:
                    if k < off + n:
                        return xt[:, k - off, hl, :]
                raise IndexError(k)

            d1 = dum.tile([1, 1], F32)

            # ---------------- Phase 1: encoder GEMM (hi/lo bf16) ----------------
            with (
                tc.tile_pool(name="wts", bufs=9) as wts,
                tc.tile_pool(name="encp", bufs=1, space="PSUM") as encp,
            ):
                pe = encp.tile([B, NCOLS + 1], F32)
                nc.tensor.matmul(
                    d1, lhsT=xts[0][2][:, 0, 0, 0:1], rhs=xts[0][2][:, 0, 0, 0:1],
                    start=True, stop=True, skip_group_check=True,
                )
                for ci in range(NCH):
                    w_c = wts.tile([128, TPC, 2, NCOLS], BF16)
                    eng = (nc.sync, nc.scalar, nc.gpsimd)[ci % 3]
                    eng.dma_start(out=w_c, in_=w_p[ci])
                    for t in range(TPC):
                        k = ci * TPC + t
                        nc.tensor.matmul(
                            pe[:, 0:NCOLS], lhsT=xap(k, 0), rhs=w_c[:, t, 0, :],
                            start=(k == 0), stop=False,
                        )
                        nc.tensor.matmul(
                            pe[:, 0:NCOLS], lhsT=xap(k, 0), rhs=w_c[:, t, 1, :],
                            start=False, stop=False,
                        )
                        nc.tensor.matmul(
                            pe[:, 0:NCOLS], lhsT=xap(k, 1), rhs=w_c[:, t, 0, :],
                            start=False, stop=False,
                        )
                # bias (scaled 1/8) + constant 1/8 lane in col 411, fp32
                nc.tensor.matmul(
                    d1, lhsT=b_sb[0:1, 0:1], rhs=b_sb[0:1, 0:1],
                    start=True, stop=True, skip_group_check=True,
                )
                nc.tensor.matmul(
                    pe, lhsT=ones1, rhs=b_sb, start=False, stop=True,
                )
                lat1 = latp.tile([B, NCOLS + 1], F32)
                nc.vector.tensor_copy(out=lat1, in_=pe)
                nc.sync.dma_start(out=ar_in[:, :], in_=lat1)

            nc.gpsimd.collective_compute(
                "AllGather",
                ALU.bypass,
                replica_groups=[list(range(NCORES))],
                ins=[ar_in.ap().opt()],
                outs=[ag_out.ap().opt()],
            )
            parts = latp.tile([B, NCORES, NCOLS + 1], F32)
            ag_v = ag_out.ap().rearrange("(c b) n -> b c n", b=B)
            nc.sync.dma_start(out=parts[:, 0:4, :], in_=ag_v[:, 0:4, :])
            nc.scalar.dma_start(out=parts[:, 4:8, :], in_=ag_v[:, 4:8, :])
            nc.vector.tensor_tensor(
                out=parts[:, 0:4, :], in0=parts[:, 0:4, :], in1=parts[:, 4:8, :],
                op=ALU.add,
            )
            nc.vector.tensor_tensor(
                out=parts[:, 0:2, :], in0=parts[:, 0:2, :], in1=parts[:, 2:4, :],
                op=ALU.add,
            )
            nc.vector.tensor_tensor(
                out=lat[:, 0:NCOLS + 1], in0=parts[:, 0, :], in1=parts[:, 1, :],
                op=ALU.add,
            )

            # ---------------- Phase 1.5: transpose shape params on PE ----------
            with tc.tile_pool(name="trps", bufs=1, space="PSUM") as trps:
                trp = trps.tile([128, 4, B], F32)
                nc.tensor.matmul(
                    d1, lhsT=eye_sb[0:1, 0:1], rhs=eye_sb[0:1, 0:1],
                    start=True, stop=True, skip_group_check=True,
                )
                for kt in range(3):
                    nc.tensor.matmul(
                        trp[:, kt, :], lhsT=lat[:, kt * 128:(kt + 1) * 128],
                        rhs=eye_sb, is_transpose=True,
                        start=True, stop=True, skip_group_check=True,
                    )
                nc.tensor.matmul(
                    trp[0:32, 3, :], lhsT=lat[:, 384:416],
                    rhs=eye_sb, is_transpose=True,
                    start=True, stop=True, skip_group_check=True,
                )
                spT = latp.tile([128, 4, B], F32)
                nc.scalar.copy(out=spT, in_=trp)

            # ---------------- Phase 2: blendshape GEMM (V-sharded) -------------
            vpre = planep.tile([B, N2], F32)
            NSPL2 = [(0, 512), (512, 512), (1024, 512), (1536, N2 - 1536)]
            with tc.tile_pool(name="p2ps", bufs=1, space="PSUM") as p2ps:
                pvs = [
                    p2ps.tile([B, n], F32, name=f"pv{j}", tag=f"pv{j}")
                    for j, (_, n) in enumerate(NSPL2)
                ]
                nc.tensor.matmul(
                    d1, lhsT=basis_sb[0:1, 0, 0:1], rhs=basis_sb[0:1, 0, 0:1],
                    start=True, stop=True, skip_group_check=True,
                )
                for j, (n0, n) in enumerate(NSPL2):
                    for kt in range(4):
                        rows = 128 if kt < 3 else 32
                        nc.tensor.matmul(
                            pvs[j],
                            lhsT=spT[0:rows, kt, :],
                            rhs=basis_sb[0:rows, kt, n0:n0 + n],
                            start=(kt == 0),
                            stop=(kt == 3),
                        )

                # eyeball rotation inputs (copies allowed on GpSimd)
                aa2 = geop.tile([128, 3], F32)
                nc.gpsimd.memset(aa2, 0.0)
                nc.gpsimd.tensor_copy(out=aa2[0:B, 0:2], in_=lat[:, P_LR:P_LR + 2])
                nc.sync.dma_start(out=aa2[B:128, 0:2], in_=lat[:, P_RR:P_RR + 2])

                # face rotation (DVE, overlaps the GEMM)
                g = Geo(nc, geop)
                fD, fMI, fPL = axis_angle_R(
                    nc, g, lat[:, P_ROT:P_ROT + 3], "f_", halfpi[:B, :]
                )
                fs = g.t()
                nc.vector.tensor_scalar_add(
                    out=fs, in0=lat[:, P_SC:P_SC + 1], scalar1=1.0
                )
                RsD = geop.tile([B, 3], F32)
                RsMI = geop.tile([B, 3], F32)
                RsPL = geop.tile([B, 3], F32)
                nc.vector.tensor_scalar_mul(out=RsD, in0=fD, scalar1=fs)
                nc.vector.tensor_scalar_mul(out=RsMI, in0=fMI, scalar1=fs)
                nc.vector.tensor_scalar_mul(out=RsPL, in0=fPL, scalar1=fs)

                def rs(l, i):
                    return r_at(RsD, RsMI, RsPL, l, i)

                # eyeball rotations: DVE work that needs only lat; fills the
                # DVE idle while the PE runs the blendshape GEMM
                g2 = Geo(nc, geop, rows=128)
                eD, eMI, ePL = axis_angle_R(nc, g2, aa2, "e_", halfpi)
                gz = geop.tile([128, 3], F32)
                nc.vector.tensor_scalar_mul(
                    out=gz[:, 0:1], in0=r_at(eD, eMI, ePL, 2, 0), scalar1=GAZE_DIR
                )
                nc.vector.tensor_scalar_mul(
                    out=gz[:, 1:2], in0=r_at(eD, eMI, ePL, 2, 1), scalar1=GAZE_DIR
                )
                nc.vector.tensor_scalar_mul(
                    out=gz[:, 2:3], in0=r_at(eD, eMI, ePL, 2, 2), scalar1=GAZE_DIR
                )
                rg64 = geop.tile([B, 3], F32)
                nc.sync.dma_start(out=rg64, in_=gz[B:128, :])
                lg = [gz[0:B, i:i + 1] for i in range(3)]
                rg = [rg64[:, i:i + 1] for i in range(3)]

                for j, (n0, n) in enumerate([NSPL2[3], NSPL2[1], NSPL2[2], NSPL2[0]]):
                    nc.scalar.copy(out=vpre[:, n0:n0 + n], in_=pvs[NSPL2.index((n0, n))])

            # offsets: off_i = face_t_i - sum_l vms_l*Rs[l,i]
            off3 = geop.tile([B, 3], F32)
            for i in range(3):
                t = g.mul(vpre[:, 519:520], rs(0, i))
                t = g.mac(vpre[:, 520 + 519:520 + 520], rs(1, i), t)
                t = g.mac(vpre[:, 1040 + 519:1040 + 520], rs(2, i), t)
                nc.vector.tensor_tensor(
                    out=off3[:, i:i + 1], in0=lat[:, P_T + i:P_T + i + 1], in1=t,
                    op=ALU.subtract,
                )

            # rotate + translate all plane blocks
            rt = planep.tile([B, 3, PL], F32)
            for i in range(3):
                nc.vector.tensor_scalar(
                    out=rt[:, i, :], in0=vpre[:, 0:PL],
                    scalar1=rs(0, i), scalar2=off3[:, i:i + 1],
                    op0=ALU.mult, op1=ALU.add,
                )
                for l in (1, 2):
                    nc.vector.scalar_tensor_tensor(
                        out=rt[:, i, :], in0=vpre[:, l * PL:(l + 1) * PL],
                        scalar=rs(l, i),
                        in1=rt[:, i, :],
                        op0=ALU.mult, op1=ALU.add,
                    )

            lc = [rt[:, i, SL + 68:SL + 69] for i in range(3)]
            rc = [rt[:, i, SL + 69:SL + 70] for i in range(3)]

            # projection of this core's vert slice (DVE)
            with tc.tile_pool(name="imgp", bufs=1) as imgp:
                img = imgp.tile([B, 3, SL], F32)
                for i in (2, 0, 1):  # z first (feeds the clamp chain)
                    nc.vector.tensor_scalar(
                        out=img[:, i, :], in0=rt[:, 0, 0:SL],
                        scalar1=cam[:, 4 * i:4 * i + 1],
                        scalar2=cam[:, 4 * i + 3:4 * i + 4],
                        op0=ALU.mult, op1=ALU.add,
                    )
                    for l in (1, 2):
                        nc.vector.scalar_tensor_tensor(
                            out=img[:, i, :], in0=rt[:, l, 0:SL],
                            scalar=cam[:, 4 * i + l:4 * i + l + 1], in1=img[:, i, :],
                            op0=ALU.mult, op1=ALU.add,
                        )
                az_ = imgp.tile([B, SL], F32)
                nc.scalar.activation(out=az_, in_=img[:, 2, :], func=ACTF.Abs)
                nc.vector.tensor_scalar_max(out=az_, in0=az_, scalar1=1e-3)
                sg = imgp.tile([B, SL], F32)
                nc.vector.tensor_scalar(
                    out=sg, in0=img[:, 2, :], scalar1=0.0, scalar2=None, op0=ALU.is_ge
                )
                nc.vector.tensor_scalar(
                    out=sg, in0=sg, scalar1=2.0, scalar2=1.0,
                    op0=ALU.mult, op1=ALU.subtract,
                )
                nc.vector.tensor_tensor(out=sg, in0=sg, in1=az_, op=ALU.mult)
                nc.vector.reciprocal(out=az_, in_=sg)
                nc.vector.tensor_tensor(
                    out=img[:, 0, :], in0=img[:, 0, :], in1=az_, op=ALU.mult
                )
                nc.vector.tensor_tensor(
                    out=img[:, 1, :], in0=img[:, 1, :], in1=az_, op=ALU.mult
                )

                # vert + img outputs can ship while the tail is computed
                nc.sync.dma_start(out=out_p[:, :, 0:SL], in_=rt[:, :, 0:SL])
                nc.scalar.dma_start(out=out_p[:, :, SL:2 * SL], in_=img)
                nc.sync.dma_start(
                    out=out_p[:, :, 2 * SL:2 * SL + 71], in_=rt[:, :, SL:SL + 71]
                )

                # tail block ge[:, i, j]: gp_l gp_r gp_mid far_l far_r lg rg dist
                ge = geop.tile([B, 3, 8], F32)
                for i in range(3):
                    # independent pieces off the DVE critical chain
                    nc.vector.scalar_tensor_tensor(
                        out=ge[:, i, 3:4], in0=lg[i], scalar=1000.0,
                        in1=lc[i], op0=ALU.mult, op1=ALU.add,
                    )
                    nc.vector.scalar_tensor_tensor(
                        out=ge[:, i, 4:5], in0=rg[i], scalar=1000.0,
                        in1=rc[i], op0=ALU.mult, op1=ALU.add,
                    )
                    nc.gpsimd.tensor_copy(out=ge[:, i, 5:6], in_=lg[i])
                    nc.gpsimd.tensor_copy(out=ge[:, i, 6:7], in_=rg[i])

                # gaze intersection: with |lg|=|rg|=1 the Cramer solve
                # collapses to c=lg.rg, det=1-c^2, sol0=(d.lg - c d.rg)/det,
                # sol1=(c d.lg - d.rg)/det  (triple-product expansion of
                # A=[lg, -rg, rg x lg])
                d3 = geop.tile([B, 3, 1], F32)
                nc.vector.tensor_tensor(
                    out=d3, in0=rt[:, :, SL + 69:SL + 70],
                    in1=rt[:, :, SL + 68:SL + 69], op=ALU.subtract,
                )
                lg3 = gz[0:B, :]
                rg3 = rg64
                dv = d3[:, :, 0]
                cw = g.t(3)
                nc.vector.tensor_tensor(out=cw, in0=lg3, in1=rg3, op=ALU.mult)
                c_ = g.t()
                nc.vector.tensor_reduce(out=c_, in_=cw, axis=AX.X, op=ALU.add)
                pw = g.t(3)
                nc.vector.tensor_tensor(out=pw, in0=dv, in1=lg3, op=ALU.mult)
                p_ = g.t()
                nc.vector.tensor_reduce(out=p_, in_=pw, axis=AX.X, op=ALU.add)
                qw = g.t(3)
                nc.vector.tensor_tensor(out=qw, in0=dv, in1=rg3, op=ALU.mult)
                q_ = g.t()
                nc.vector.tensor_reduce(out=q_, in_=qw, axis=AX.X, op=ALU.add)
                csq = g.mul(c_, c_)
                det = g.t()
                nc.vector.tensor_scalar(
                    out=det, in0=csq, scalar1=-1.0, scalar2=1.0,
                    op0=ALU.mult, op1=ALU.add,
                )
                rdet = g.t()
                nc.vector.reciprocal(out=rdet, in_=det)
                sol0 = g.mul(g.sub(p_, g.mul(c_, q_)), rdet)
                sol1 = g.mul(g.sub(g.mul(c_, p_), q_), rdet)

                gpl = geop.tile([B, 3], F32)
                gpr = geop.tile([B, 3], F32)
                for i in range(3):
                    nc.vector.scalar_tensor_tensor(
                        out=gpl[:, i:i + 1], in0=lg[i], scalar=sol0,
                        in1=lc[i], op0=ALU.mult, op1=ALU.add,
                    )
                    nc.vector.scalar_tensor_tensor(
                        out=gpr[:, i:i + 1], in0=rg[i], scalar=sol1,
                        in1=rc[i], op0=ALU.mult, op1=ALU.add,
                    )
                    nc.vector.tensor_copy(out=ge[:, i, 0:1], in_=gpl[:, i:i + 1])
                    nc.vector.tensor_copy(out=ge[:, i, 1:2], in_=gpr[:, i:i + 1])
                    o = g.add(gpl[:, i:i + 1], gpr[:, i:i + 1])
                    nc.vector.tensor_scalar_mul(out=ge[:, i, 2:3], in0=o, scalar1=0.5)
                dff = geop.tile([B, 3], F32)
                nc.vector.tensor_tensor(out=dff, in0=gpl, in1=gpr, op=ALU.subtract)
                nc.vector.tensor_tensor(out=dff, in0=dff, in1=dff, op=ALU.mult)
                d2 = g.t()
                nc.vector.tensor_reduce(out=d2, in_=dff, axis=AX.X, op=ALU.add)
                dist = g.t()
                nc.scalar.activation(out=dist, in_=d2, func=ACTF.Sqrt)
                for i in range(3):
                    nc.scalar.copy(out=ge[:, i, 7:8], in_=dist)

                nc.scalar.dma_start(out=out_p[:, :, 2 * SL + 71:2 * SL + 79], in_=ge)
    _legalize_waits(nc)
    return nc


def _prep(inputs):
    f32 = np.float32
    x = np.ascontiguousarray(inputs["x"].reshape(B, DIN), dtype=f32)
    W = np.asarray(inputs["enc_W"], dtype=f32)
    Wp = np.concatenate([W[:, :400], W[:, 545:556]], axis=1)  # [DIN, 411]
    enc_b = np.asarray(inputs["enc_b"], dtype=f32)
    bp = np.concatenate([enc_b[:400], enc_b[545:556]])
    bvec = np.concatenate(
        [bp / NCORES, np.array([1.0 / NCORES], f32)]
    ).reshape(1, NCOLS + 1).astype(f32)
    tmpl = np.asarray(inputs["v_template"], dtype=f32)  # [V, 3]
    basis = np.asarray(inputs["shape_basis"], dtype=f32)  # [400, V, 3]
    cam = np.ascontiguousarray(
        np.asarray(inputs["camera_parameters"], dtype=f32).reshape(B, 12)
    )
    lm = np.asarray(inputs["landmarks"])
    mlm = np.asarray(inputs["masked_landmarks"])
    fmask = np.asarray(inputs["face_mask"])
    lmask = np.asarray(inputs["left_eyeball_mask"])
    rmask = np.asarray(inputs["right_eyeball_mask"])
    fl_idx = fmask[mlm]  # verts behind the 68 output landmarks
    idx4 = lm[np.array([19, 22, 25, 28])]
    idx2 = lm[np.array([14, 18])]

    # synthetic extra columns [400, 72, 3] / [72, 3]
    ex_b = np.concatenate([
        basis[:, fl_idx, :],
        basis[:, lmask, :].mean(axis=1, keepdims=True),
        basis[:, rmask, :].mean(axis=1, keepdims=True),
        (basis[:, idx4, :].mean(axis=1, keepdims=True)
         + basis[:, idx2, :].mean(axis=1, keepdims=True)) / 2.0,
        basis.mean(axis=1, keepdims=True),
    ], axis=1)
    ex_t = np.concatenate([
        tmpl[fl_idx],
        tmpl[lmask].mean(axis=0, keepdims=True),
        tmpl[rmask].mean(axis=0, keepdims=True),
        (tmpl[idx4].mean(axis=0, keepdims=True)
         + tmpl[idx2].mean(axis=0, keepdims=True)) / 2.0,
        tmpl.mean(axis=0, keepdims=True),
    ], axis=0)

    eye = np.eye(B, dtype=f32)
    in_maps = []
    for c in range(NCORES):
        k0 = c * KSH
        xs = x[:, k0:k0 + KSH].T  # [KSH, B] f32
        xh = xs.astype(BF)
        xl = (xs - xh.astype(f32)).astype(BF)
        xw = np.ascontiguousarray(
            np.stack([
                xh.reshape(KT, 128, B).transpose(1, 0, 2),
                xl.reshape(KT, 128, B).transpose(1, 0, 2),
            ], axis=2)
        )  # [128, KT, 2, B] bf16
        ws = Wp[k0:k0 + KSH]  # [KSH, 411] f32
        wh = ws.astype(BF)
        wl = (ws - wh.astype(f32)).astype(BF)
        wch = np.ascontiguousarray(
            np.stack([
                wh.reshape(NCH, TPC, 128, NCOLS).transpose(0, 2, 1, 3),
                wl.reshape(NCH, TPC, 128, NCOLS).transpose(0, 2, 1, 3),
            ], axis=3)
        )  # [NCH, 128, TPC, 2, 411] bf16

        lo = c * SL
        verts = fmask[lo:min(lo + SL, VM)]
        nsl = len(verts)
        blk = np.zeros((400, N2), f32)
        trow = np.zeros(N2, f32)
        for l in range(3):
            blk[:, l * PL:l * PL + nsl] = basis[:, verts, l]
            blk[:, l * PL + SL:l * PL + SL + 72] = ex_b[:, :, l]
            trow[l * PL:l * PL + nsl] = tmpl[verts, l]
            trow[l * PL + SL:l * PL + SL + 72] = ex_t[:, l]
        bh = np.zeros((128, 4, N2), f32)
        for kt in range(3):
            bh[:, kt, :] = blk[kt * 128:(kt + 1) * 128]
        bh[0:16, 3, :] = blk[384:400]
        bh[27, 3, :] = trow  # coefficient = exact 1.0 from AR col 411
        in_maps.append({
            "xw": xw,
            "wch": wch,
            "bvec": bvec,
            "basis": np.ascontiguousarray(bh),
            "cam": cam,
            "eye": eye,
        })
    return in_maps


def _run(inputs, trace=False):
    in_maps = _prep(inputs)
    nc = build_graph()
    res = run_bass_kernel_spmd(
        nc, in_maps, core_ids=list(range(NCORES)), trace=trace
    )
    full = np.empty((B, 3, NOUT), np.float32)
    for c in range(NCORES):
        r = res.results[c]["out"]  # [B, 3, 975]
        lo = c * SL
        w = min(SL, VM - lo)
        full[:, :, lo:lo + w] = r[:, :, 0:w]
        full[:, :, VM + lo:VM + lo + w] = r[:, :, SL:SL + w]
    r0 = res.results[0]["out"]
    full[:, :, 2 * VM:NOUT] = r0[:, :, 2 * SL:2 * SL + 79]
    return np.ascontiguousarray(full.transpose(0, 2, 1)), res


def kernel(**inputs):
    out, _ = _run(inputs, trace=False)
    return out


# revision 20
# speedup vs baseline: 1.1557x; 1.0102x over previous
"""Trainium2 Bass kernel for nn_Autoencoder_65223373357102 (FLAME-style autoencoder).

Strategy (v6):
  Phase 1 (8-way tensor parallel): encoder GEMM sharded along K, W packed to
  the 411 *used* latent columns. The fp32 GEMM is decomposed into three bf16
  passes (x_hi*W_hi + x_hi*W_lo + x_lo*W_hi, fp32 PSUM accumulation): bf16
  products are exact in fp32, so the latent error is ~4e-6 relative - inside
  the ~1e-5 budget set by the z-clamped projection - while the PE runs 1
  cycle/row instead of fp32's 4. x (hi+lo) is SBUF-resident; W streams in 21
  pre-tiled contiguous chunks on two DMA queues, deep-buffered so the NRT
  start barrier overlaps prefetch. Bias (scaled 1/8) and a constant 1/8 lane
  (col 411) are folded into the PSUM accumulation; the AllReduce of [64,412]
  then yields latent + an exact 1.0 in col 411 that phase 2 uses as the
  template coefficient.
  Phase 2 (8-way vertex parallel): each core computes only its 448 of the 3500
  face verts plus 72 synthetic columns (68 landmarks, l/r eye means, face
  centre, vmean) via an fp32 [64,400+]@[400+,1560] GEMM from host-gathered
  basis columns. Everything the reference does to the eye vertex slices is
  dead code w.r.t. the output (only the eye means and gaze rotations survive).
  shape_p is transposed on the PE (identity matmul). The latent AllReduce is
  an AllGather + local tree-reduce (fewer RDH steps), preceded by a warm-up
  AllGather that absorbs the collective firmware wakeup (~11us -> ~1us
  trigger delay). The gaze solve uses the closed form for unit gaze vectors
  (det = 1 - (lg.rg)^2) instead of a general 3x3 Cramer chain. Per-core
  output [64,3,975] is stitched to the full [64,7079,3] on the host.
"""
import sys
import types

sys.path.insert(0, "/opt/trn_rl_repo")

import numpy as np
import ml_dtypes

BF = ml_dtypes.bfloat16


def _ensure_ntff_hook():
    """Provide antenv.axon_hooks + install the ctypes NTFF profile hook so
    run_bass_kernel_spmd(trace=True) can pull a neuron-profile under axon."""
    name = "antenv.axon_hooks"
    if name not in sys.modules:
        mod = types.ModuleType(name)
        mod._HOOK = None

        def set_axon_ntff_profile_hook(hook):
            mod._HOOK = hook

        def get_axon_ntff_profile_hook():
            return mod._HOOK

        mod.set_axon_ntff_profile_hook = set_axon_ntff_profile_hook
        mod.get_axon_ntff_profile_hook = get_axon_ntff_profile_hook
        sys.modules[name] = mod
        try:
            import antenv

            antenv.axon_hooks = mod
        except ImportError:
            pass
    mod = sys.modules[name]
    if mod.get_axon_ntff_profile_hook() is None:
        try:
            from trn_agent_boot.trn_boot import _ntff_profile_via_ctypes

            hook = _ntff_profile_via_ctypes("/opt/axon/libaxon_pjrt.so")
            if hook is not None:
                mod.set_axon_ntff_profile_hook(hook)
        except Exception:
            pass


_ensure_ntff_hook()

from concourse import bass, mybir, tile
from concourse.bass_utils import run_bass_kernel_spmd

F32 = mybir.dt.float32
BF16 = mybir.dt.bfloat16
ALU = mybir.AluOpType
ACTF = mybir.ActivationFunctionType
AX = mybir.AxisListType

B = 64
V = 5023
VM = 3500
LAT = 556
DIN = 3 * 224 * 224  # 150528
NCORES = 8
KSH = DIN // NCORES  # 18816
KT = KSH // 128  # 147 k-tiles
TPC = 7  # k-tiles per W chunk
NCH = KT // TPC  # 21 chunks
NCOLS = 411  # packed latent cols: 0:400 + 545:556
NOUT = 2 * VM + 68 + 11  # 7079
SL = 448  # verts per core (last core: 364 real + pad)
PL = SL + 68 + 4  # per-plane block: slice, fl, lme, rme, fc, vmean = 520
N2 = 3 * PL  # 1560
GAZE_DIR = -1.0
HALF_PI = 1.5707963267948966
# packed pose col offsets (orig 545:556 -> packed 400:411)
P_ROT, P_T, P_SC, P_LR, P_RR = 400, 403, 406, 407, 409


class Geo:
    """Helper for tiny per-batch scalar ops on [rows,1] tiles."""

    _uid = [0]

    def __init__(self, nc, pool, rows=B, eng=None):
        self.nc = nc
        self.pool = pool
        self.rows = rows
        self.eng = eng if eng is not None else nc.vector

    def t(self, cols=1):
        Geo._uid[0] += 1
        return self.pool.tile([self.rows, cols], F32, name=f"g{Geo._uid[0]}_{cols}")

    def mul(self, a, b):
        o = self.t()
        self.eng.tensor_tensor(out=o, in0=a, in1=b, op=ALU.mult)
        return o

    def add(self, a, b):
        o = self.t()
        self.eng.tensor_tensor(out=o, in0=a, in1=b, op=ALU.add)
        return o

    def sub(self, a, b):
        o = self.t()
        self.eng.tensor_tensor(out=o, in0=a, in1=b, op=ALU.subtract)
        return o

    def mac(self, a, s, acc):
        """(a * s) + acc, s is a [rows,1] AP scalar."""
        o = self.t()
        self.eng.scalar_tensor_tensor(
            out=o, in0=a, scalar=s, in1=acc, op0=ALU.mult, op1=ALU.add
        )
        return o

    def dot3(self, ax, ay, az, bx, by, bz):
        o = self.mul(ax, bx)
        o = self.mac(ay, by, o)
        o = self.mac(az, bz, o)
        return o

    def cross3(self, ax, ay, az, bx, by, bz):
        cx = self.sub(self.mul(ay, bz), self.mul(az, by))
        cy = self.sub(self.mul(az, bx), self.mul(ax, bz))
        cz = self.sub(self.mul(ax, by), self.mul(ay, bx))
        return cx, cy, cz


def axis_angle_R(nc, g, aa3, pfx, halfpi):
    R_ = g.rows
    """aa3: [rows,3] axis-angle tile -> (D, MI, PL3) [rows,3] tiles with
    R[0,0],R[1,1],R[2,2] = D[:,0..2]
    R[0,1],R[1,2],R[2,0] = MI[:,0..2]  (m - s terms)
    R[1,0],R[2,1],R[0,2] = PL3[:,0..2] (m + s terms)
    """
    pool = g.pool
    eng = g.eng
    sq = pool.tile([R_, 3], F32, name=pfx + "aaR_sq")
    eng.tensor_tensor(out=sq, in0=aa3, in1=aa3, op=ALU.mult)
    th2a = g.t()
    eng.tensor_tensor(out=th2a, in0=sq[:, 0:1], in1=sq[:, 1:2], op=ALU.add)
    th2 = g.t()
    eng.tensor_tensor(out=th2, in0=th2a, in1=sq[:, 2:3], op=ALU.add)
    theta = g.t()
    nc.scalar.activation(out=theta, in_=th2, func=ACTF.Sqrt)
    thm = g.t()
    eng.tensor_scalar_max(out=thm, in0=theta, scalar1=1e-8)
    rth = g.t()
    nc.vector.reciprocal(out=rth, in_=thm)
    axis3 = pool.tile([R_, 3], F32, name=pfx + "aaR_axis")
    eng.tensor_scalar_mul(out=axis3, in0=aa3, scalar1=rth)
    s = g.t()
    nc.scalar.activation(out=s, in_=theta, func=ACTF.Sin)
    c = g.t()
    nc.scalar.activation(out=c, in_=theta, func=ACTF.Sin, bias=halfpi)
    omc = g.t()
    eng.tensor_scalar(
        out=omc, in0=c, scalar1=-1.0, scalar2=1.0, op0=ALU.mult, op1=ALU.add
    )
    asq = pool.tile([R_, 3], F32, name=pfx + "aaR_asq")
    eng.tensor_tensor(out=asq, in0=axis3, in1=axis3, op=ALU.mult)
    dmul = pool.tile([R_, 3], F32, name=pfx + "aaR_dmul")
    eng.tensor_scalar_mul(out=dmul, in0=asq, scalar1=omc)
    D = pool.tile([R_, 3], F32, name=pfx + "aaR_D")
    eng.tensor_scalar(out=D, in0=dmul, scalar1=c, op0=ALU.add, scalar2=None)
    # m3 = (ax*ay, ay*az, az*ax) * omc ; s3 = (s*az, s*ax, s*ay)
    r1 = pool.tile([R_, 3], F32, name=pfx + "aaR_r1")
    eng.tensor_copy(out=r1[:, 0:2], in_=axis3[:, 1:3])
    eng.tensor_copy(out=r1[:, 2:3], in_=axis3[:, 0:1])
    m3 = pool.tile([R_, 3], F32, name=pfx + "aaR_m3")
    eng.tensor_tensor(out=m3, in0=axis3, in1=r1, op=ALU.mult)
    eng.tensor_scalar_mul(out=m3, in0=m3, scalar1=omc)
    sa = pool.tile([R_, 3], F32, name=pfx + "aaR_sa")
    eng.tensor_scalar_mul(out=sa, in0=axis3, scalar1=s)
    sr = pool.tile([R_, 3], F32, name=pfx + "aaR_sr")
    eng.tensor_copy(out=sr[:, 0:1], in_=sa[:, 2:3])
    eng.tensor_copy(out=sr[:, 1:3], in_=sa[:, 0:2])
    MI = pool.tile([R_, 3], F32, name=pfx + "aaR_MI")
    eng.tensor_tensor(out=MI, in0=m3, in1=sr, op=ALU.subtract)
    PL3 = pool.tile([R_, 3], F32, name=pfx + "aaR_PL")
    eng.tensor_tensor(out=PL3, in0=m3, in1=sr, op=ALU.add)
    return D, MI, PL3


def r_at(D, MI, PL3, l, i):
    """R[l,i] as a [rows,1] AP from the (D, MI, PL3) triple."""
    if l == i:
        return D[:, l:l + 1]
    m = {(0, 1): MI[:, 0:1], (1, 2): MI[:, 1:2], (2, 0): MI[:, 2:3],
         (1, 0): PL3[:, 0:1], (2, 1): PL3[:, 1:2], (0, 2): PL3[:, 2:3]}
    return m[(l, i)]


_ENG_ATTR = {
    "SP": "sync", "Pool": "gpsimd", "PE": "tensor",
    "DVE": "vector", "Activation": "scalar",
}


def _legalize_waits(nc):
    """This walrus accepts only one sync-wait slot per instruction; move extra
    waits onto same-engine NoOps inserted right before the instruction."""
    import concourse.mybir as _mybir

    def make_nop(engine):
        eng = getattr(nc, _ENG_ATTR[engine.name])
        bi = eng.nop(nofuse=True)
        mi = bi.ins
        for bb in nc.main_func.blocks:
            if bb.instructions and bb.instructions[-1].name == mi.name:
                bb.instructions.pop()
                break
        mi.engine = engine
        return mi

    for bb in nc.main_func.blocks:
        snapshot = list(bb.instructions)
        newlist = []
        changed = False
        for inst in snapshot:
            si = inst.sync_info
            waits = list(si.on_wait) if (si and si.on_wait) else []
            if (
                len(waits) > 1
                and not inst.name.startswith("barrier")
                and inst.engine is not None
                and getattr(inst.engine, "name", None) in _ENG_ATTR
            ):
                for w in waits[:-1]:
                    nop = make_nop(inst.engine)
                    nop.sync_info = _mybir.SyncInfo(on_wait=[w], on_update=[])
                    newlist.append(nop)
                inst.sync_info = _mybir.SyncInfo(
                    on_wait=[waits[-1]], on_update=list(si.on_update)
                )
                changed = True
            newlist.append(inst)
        if changed:
            bb.instructions[:] = newlist
    return nc


XPARTS = [25, 25, 25, 25, 25, 22]  # k-tile split of the resident x shard


def build_graph():
    nc = bass.Bass(target_bir_lowering=False)

    x_p = nc.declare_dram_parameter("xw", [128, KT, 2, B], BF16, isOutput=False)
    w_p = nc.declare_dram_parameter(
        "wch", [NCH, 128, TPC, 2, NCOLS], BF16, isOutput=False
    )
    b_p = nc.declare_dram_parameter("bvec", [1, NCOLS + 1], F32, isOutput=False)
    bas_p = nc.declare_dram_parameter("basis", [128, 4, N2], F32, isOutput=False)
    cam_p = nc.declare_dram_parameter("cam", [B, 12], F32, isOutput=False)
    eye_p = nc.declare_dram_parameter("eye", [B, B], F32, isOutput=False)
    out_p = nc.declare_dram_parameter("out", [B, 3, 2 * SL + 71 + 8], F32, isOutput=True)

    ar_in = nc.dram_tensor("ar_in", [B, NCOLS + 1], F32)
    ag_out = nc.dram_tensor("ag_out", [NCORES * B, NCOLS + 1], F32, addr_space="Shared")
    warm_in = nc.dram_tensor("warm_in", [1, 4], F32)
    warm_out = nc.dram_tensor("warm_out", [NCORES, 4], F32, addr_space="Shared")

    with tile.TileContext(nc) as tc:
        with (
            tc.tile_pool(name="consts", bufs=1) as consts,
            tc.tile_pool(name="xres", bufs=1) as xres,
            tc.tile_pool(name="latp", bufs=1) as latp,
            tc.tile_pool(name="geop", bufs=1) as geop,
            tc.tile_pool(name="planep", bufs=1) as planep,
            tc.tile_pool(name="dum", bufs=1, space="PSUM") as dum,
        ):
            # ---- const / prefetch loads ----
            eye_sb = consts.tile([B, B], F32)
            nc.scalar.dma_start(out=eye_sb, in_=eye_p[:, :])
            cam = consts.tile([B, 12], F32)
            nc.scalar.dma_start(out=cam, in_=cam_p[:, :])
            b_sb = consts.tile([1, NCOLS + 1], F32)
            nc.scalar.dma_start(out=b_sb, in_=b_p[:, :])
            ones1 = consts.tile([1, B], F32)
            nc.vector.memset(ones1, 1.0)
            halfpi = consts.tile([128, 1], F32)
            nc.vector.memset(halfpi, HALF_PI)
            lat = latp.tile([B, 416], F32)
            nc.vector.memset(lat, 0.0)

            # warm up the collective firmware path so the real gather below
            # does not pay first-op wakeup costs
            nc.gpsimd.collective_compute(
                "AllGather",
                ALU.bypass,
                replica_groups=[list(range(NCORES))],
                ins=[warm_in.ap().opt()],
                outs=[warm_out.ap().opt()],
            )

            # resident x shard (hi+lo), split so PE can start after part 0
            xts = []
            off = 0
            for pi, n in enumerate(XPARTS):
                xt = xres.tile([128, n, 2, B], BF16, name=f"xt{pi}")
                nc.gpsimd.dma_start(out=xt, in_=x_p[:, off:off + n, :, :])
                xts.append((off, n, xt))
                off += n
            # phase-2 basis block (prefetch; lands during phase 1)
            basis_sb = planep.tile([128, 4, N2], F32)
            nc.gpsimd.dma_start(out=basis_sb, in_=bas_p[:, :, :])

            def xap(k, hl):
                for off, n, xt in xts:
                    if k < off + n:
                        return xt[:, k - off, hl, :]
                raise IndexError(k)

            d1 = dum.tile([1, 1], F32)

            # ---------------- Phase 1: encoder GEMM (hi/lo bf16) ----------------
            with (
                tc.tile_pool(name="wts", bufs=9) as wts,
                tc.tile_pool(name="encp", bufs=1, space="PSUM") as encp,
            ):
                pe = encp.tile([B, NCOLS + 1], F32)
                nc.tensor.matmul(
                    d1, lhsT=xts[0][2][:, 0, 0, 0:1], rhs=xts[0][2][:, 0, 0, 0:1],
                    start=True, stop=True, skip_group_check=True,
                )
                for ci in range(NCH):
                    w_c = wts.tile([128, TPC, 2, NCOLS], BF16)
                    eng = (nc.sync, nc.scalar, nc.gpsimd)[ci % 3]
                    eng.dma_start(out=w_c, in_=w_p[ci])
                    for t in range(TPC):
                        k = ci * TPC + t
                        nc.tensor.matmul(
                            pe[:, 0:NCOLS], lhsT=xap(k, 0), rhs=w_c[:, t, 0, :],
                            start=(k == 0), stop=False,
                        )
                        nc.tensor.matmul(
                            pe[:, 0:NCOLS], lhsT=xap(k, 0), rhs=w_c[:, t, 1, :],
                            start=False, stop=False,
                        )
                        nc.tensor.matmul(
                            pe[:, 0:NCOLS], lhsT=xap(k, 1), rhs=w_c[:, t, 0, :],
                            start=False, stop=False,
                        )
                # bias (scaled 1/8) + constant 1/8 lane in col 411, fp32
                nc.tensor.matmul(
                    d1, lhsT=b_sb[0:1, 0:1], rhs=b_sb[0:1, 0:1],
                    start=True, stop=True, skip_group_check=True,
                )
                nc.tensor.matmul(
                    pe, lhsT=ones1, rhs=b_sb, start=False, stop=True,
                )
                lat1 = latp.tile([B, NCOLS + 1], F32)
                nc.vector.tensor_copy(out=lat1, in_=pe)
                nc.sync.dma_start(out=ar_in[:, :], in_=lat1)

            nc.gpsimd.collective_compute(
                "AllGather",
                ALU.bypass,
                replica_groups=[list(range(NCORES))],
                ins=[ar_in.ap().opt()],
                outs=[ag_out.ap().opt()],
            )
            parts = latp.tile([B, NCORES, NCOLS + 1], F32)
            ag_v = ag_out.ap().rearrange("(c b) n -> b c n", b=B)
            nc.sync.dma_start(out=parts[:, 0:4, :], in_=ag_v[:, 0:4, :])
            nc.scalar.dma_start(out=parts[:, 4:8, :], in_=ag_v[:, 4:8, :])
            nc.vector.tensor_tensor(
                out=parts[:, 0:4, :], in0=parts[:, 0:4, :], in1=parts[:, 4:8, :],
                op=ALU.add,
            )
            nc.vector.tensor_tensor(
                out=parts[:, 0:2, :], in0=parts[:, 0:2, :], in1=parts[:, 2:4, :],
                op=ALU.add,
            )
            nc.vector.tensor_tensor(
                out=lat[:, 0:NCOLS + 1], in0=parts[:, 0, :], in1=parts[:, 1, :],
                op=ALU.add,
            )

            # ---------------- Phase 1.5: transpose shape params on PE ----------
            with tc.tile_pool(name="trps", bufs=1, space="PSUM") as trps:
                trp = trps.tile([128, 4, B], F32)
                nc.tensor.matmul(
                    d1, lhsT=eye_sb[0:1, 0:1], rhs=eye_sb[0:1, 0:1],
                    start=True, stop=True, skip_group_check=True,
                )
                for kt in range(3):
                    nc.tensor.matmul(
                        trp[:, kt, :], lhsT=lat[:, kt * 128:(kt + 1) * 128],
                        rhs=eye_sb, is_transpose=True,
                        start=True, stop=True, skip_group_check=True,
                    )
                nc.tensor.matmul(
                    trp[0:32, 3, :], lhsT=lat[:, 384:416],
                    rhs=eye_sb, is_transpose=True,
                    start=True, stop=True, skip_group_check=True,
                )
                spT = latp.tile([128, 4, B], F32)
                nc.scalar.copy(out=spT, in_=trp)

            # ---------------- Phase 2: blendshape GEMM (V-sharded) -------------
            vpre = planep.tile([B, N2], F32)
            NSPL2 = [(0, 512), (512, 512), (1024, 512), (1536, N2 - 1536)]
            with tc.tile_pool(name="p2ps", bufs=1, space="PSUM") as p2ps:
                pvs = [
                    p2ps.tile([B, n], F32, name=f"pv{j}", tag=f"pv{j}")
                    for j, (_, n) in enumerate(NSPL2)
                ]
                nc.tensor.matmul(
                    d1, lhsT=basis_sb[0:1, 0, 0:1], rhs=basis_sb[0:1, 0, 0:1],
                    start=True, stop=True, skip_group_check=True,
                )
                # bank order 1,2,3,0: off3 needs the vmean cols (banks
                # 1-3), so those finish while bank 0 is still on the PE
                for j in (1, 2, 3, 0):
                    n0, n = NSPL2[j]
                    for kt in range(4):
                        rows = 128 if kt < 3 else 32
                        nc.tensor.matmul(
                            pvs[j],
                            lhsT=spT[0:rows, kt, :],
                            rhs=basis_sb[0:rows, kt, n0:n0 + n],
                            start=(kt == 0),
                            stop=(kt == 3),
                        )

                # eyeball rotation inputs (copies allowed on GpSimd)
                aa2 = geop.tile([128, 3], F32)
                nc.gpsimd.memset(aa2, 0.0)
                nc.gpsimd.tensor_copy(out=aa2[0:B, 0:2], in_=lat[:, P_LR:P_LR + 2])
                nc.sync.dma_start(out=aa2[B:128, 0:2], in_=lat[:, P_RR:P_RR + 2])

                # face rotation (DVE, overlaps the GEMM)
                g = Geo(nc, geop)
                fD, fMI, fPL = axis_angle_R(
                    nc, g, lat[:, P_ROT:P_ROT + 3], "f_", halfpi[:B, :]
                )
                fs = g.t()
                nc.vector.tensor_scalar_add(
                    out=fs, in0=lat[:, P_SC:P_SC + 1], scalar1=1.0
                )
                RsD = geop.tile([B, 3], F32)
                RsMI = geop.tile([B, 3], F32)
                RsPL = geop.tile([B, 3], F32)
                nc.vector.tensor_scalar_mul(out=RsD, in0=fD, scalar1=fs)
                nc.vector.tensor_scalar_mul(out=RsMI, in0=fMI, scalar1=fs)
                nc.vector.tensor_scalar_mul(out=RsPL, in0=fPL, scalar1=fs)

                def rs(l, i):
                    return r_at(RsD, RsMI, RsPL, l, i)

                # eyeball rotations: DVE work that needs only lat; fills the
                # DVE idle while the PE runs the blendshape GEMM
                g2 = Geo(nc, geop, rows=128)
                eD, eMI, ePL = axis_angle_R(nc, g2, aa2, "e_", halfpi)
                gz = geop.tile([128, 3], F32)
                nc.vector.tensor_scalar_mul(
                    out=gz[:, 0:1], in0=r_at(eD, eMI, ePL, 2, 0), scalar1=GAZE_DIR
                )
                nc.vector.tensor_scalar_mul(
                    out=gz[:, 1:2], in0=r_at(eD, eMI, ePL, 2, 1), scalar1=GAZE_DIR
                )
                nc.vector.tensor_scalar_mul(
                    out=gz[:, 2:3], in0=r_at(eD, eMI, ePL, 2, 2), scalar1=GAZE_DIR
                )
                rg64 = geop.tile([B, 3], F32)
                nc.sync.dma_start(out=rg64, in_=gz[B:128, :])
                lg = [gz[0:B, i:i + 1] for i in range(3)]
                rg = [rg64[:, i:i + 1] for i in range(3)]

                for j in (1, 2, 3, 0):
                    n0, n = NSPL2[j]
                    nc.scalar.copy(out=vpre[:, n0:n0 + n], in_=pvs[j])

            # offsets: off_i = face_t_i - sum_l vms_l*Rs[l,i]
            off3 = geop.tile([B, 3], F32)
            for i in range(3):
                t = g.mul(vpre[:, 519:520], rs(0, i))
                t = g.mac(vpre[:, 520 + 519:520 + 520], rs(1, i), t)
                t = g.mac(vpre[:, 1040 + 519:1040 + 520], rs(2, i), t)
                nc.vector.tensor_tensor(
                    out=off3[:, i:i + 1], in0=lat[:, P_T + i:P_T + i + 1], in1=t,
                    op=ALU.subtract,
                )

            # rotate + translate all plane blocks
            rt = planep.tile([B, 3, PL], F32)
            for i in range(3):
                nc.vector.tensor_scalar(
                    out=rt[:, i, :], in0=vpre[:, 0:PL],
                    scalar1=rs(0, i), scalar2=off3[:, i:i + 1],
                    op0=ALU.mult, op1=ALU.add,
                )
                for l in (1, 2):
                    nc.vector.scalar_tensor_tensor(
                        out=rt[:, i, :], in0=vpre[:, l * PL:(l + 1) * PL],
                        scalar=rs(l, i),
                        in1=rt[:, i, :],
                        op0=ALU.mult, op1=ALU.add,
                    )

            lc = [rt[:, i, SL + 68:SL + 69] for i in range(3)]
            rc = [rt[:, i, SL + 69:SL + 70] for i in range(3)]

            # projection of this core's vert slice (DVE)
            with tc.tile_pool(name="imgp", bufs=1) as imgp:
                img = imgp.tile([B, 3, SL], F32)
                for i in (2, 0, 1):  # z first (feeds the clamp chain)
                    nc.vector.tensor_scalar(
                        out=img[:, i, :], in0=rt[:, 0, 0:SL],
                        scalar1=cam[:, 4 * i:4 * i + 1],
                        scalar2=cam[:, 4 * i + 3:4 * i + 4],
                        op0=ALU.mult, op1=ALU.add,
                    )
                    for l in (1, 2):
                        nc.vector.scalar_tensor_tensor(
                            out=img[:, i, :], in0=rt[:, l, 0:SL],
                            scalar=cam[:, 4 * i + l:4 * i + l + 1], in1=img[:, i, :],
                            op0=ALU.mult, op1=ALU.add,
                        )
                az_ = imgp.tile([B, SL], F32)
                nc.scalar.activation(out=az_, in_=img[:, 2, :], func=ACTF.Abs)
                nc.vector.tensor_scalar_max(out=az_, in0=az_, scalar1=1e-3)
                sg = imgp.tile([B, SL], F32)
                nc.vector.tensor_scalar(
                    out=sg, in0=img[:, 2, :], scalar1=0.0, scalar2=None, op0=ALU.is_ge
                )
                nc.vector.tensor_scalar(
                    out=sg, in0=sg, scalar1=2.0, scalar2=1.0,
                    op0=ALU.mult, op1=ALU.subtract,
                )
                nc.vector.tensor_tensor(out=sg, in0=sg, in1=az_, op=ALU.mult)
                nc.vector.reciprocal(out=az_, in_=sg)
                nc.vector.tensor_tensor(
                    out=img[:, 0, :], in0=img[:, 0, :], in1=az_, op=ALU.mult
                )
                nc.vector.tensor_tensor(
                    out=img[:, 1, :], in0=img[:, 1, :], in1=az_, op=ALU.mult
                )

                # vert + img outputs can ship while the tail is computed
                nc.sync.dma_start(out=out_p[:, :, 0:SL], in_=rt[:, :, 0:SL])
                nc.scalar.dma_start(out=out_p[:, :, SL:2 * SL], in_=img)
                nc.sync.dma_start(
                    out=out_p[:, :, 2 * SL:2 * SL + 71], in_=rt[:, :, SL:SL + 71]
                )

                # tail block ge[:, i, j]: gp_l gp_r gp_mid far_l far_r lg rg dist
                ge = geop.tile([B, 3, 8], F32)
                for i in range(3):
                    # independent pieces off the DVE critical chain
                    nc.vector.scalar_tensor_tensor(
                        out=ge[:, i, 3:4], in0=lg[i], scalar=1000.0,
                        in1=lc[i], op0=ALU.mult, op1=ALU.add,
                    )
                    nc.vector.scalar_tensor_tensor(
                        out=ge[:, i, 4:5], in0=rg[i], scalar=1000.0,
                        in1=rc[i], op0=ALU.mult, op1=ALU.add,
                    )
                    nc.gpsimd.tensor_copy(out=ge[:, i, 5:6], in_=lg[i])
                    nc.gpsimd.tensor_copy(out=ge[:, i, 6:7], in_=rg[i])

                # gaze intersection: with |lg|=|rg|=1 the Cramer solve
                # collapses to c=lg.rg, det=1-c^2, sol0=(d.lg - c d.rg)/det,
                # sol1=(c d.lg - d.rg)/det  (triple-product expansion of
                # A=[lg, -rg, rg x lg])
                d3 = geop.tile([B, 3, 1], F32)
                nc.vector.tensor_tensor(
                    out=d3, in0=rt[:, :, SL + 69:SL + 70],
                    in1=rt[:, :, SL + 68:SL + 69], op=ALU.subtract,
                )
                lg3 = gz[0:B, :]
                rg3 = rg64
                dv = d3[:, :, 0]
                cw = g.t(3)
                nc.vector.tensor_tensor(out=cw, in0=lg3, in1=rg3, op=ALU.mult)
                c_ = g.t()
                nc.vector.tensor_reduce(out=c_, in_=cw, axis=AX.X, op=ALU.add)
                pw = g.t(3)
                nc.vector.tensor_tensor(out=pw, in0=dv, in1=lg3, op=ALU.mult)
                p_ = g.t()
                nc.vector.tensor_reduce(out=p_, in_=pw, axis=AX.X, op=ALU.add)
                qw = g.t(3)
                nc.vector.tensor_tensor(out=qw, in0=dv, in1=rg3, op=ALU.mult)
                q_ = g.t()
                nc.vector.tensor_reduce(out=q_, in_=qw, axis=AX.X, op=ALU.add)
                csq = g.mul(c_, c_)
                det = g.t()
                nc.vector.tensor_scalar(
                    out=det, in0=csq, scalar1=-1.0, scalar2=1.0,
                    op0=ALU.mult, op1=ALU.add,
                )
                rdet = g.t()
                nc.vector.reciprocal(out=rdet, in_=det)
                sol0 = g.mul(g.sub(p_, g.mul(c_, q_)), rdet)
                sol1 = g.mul(g.sub(g.mul(c_, p_), q_), rdet)

                gpl = geop.tile([B, 3], F32)
                gpr = geop.tile([B, 3], F32)
                for i in range(3):
                    nc.vector.scalar_tensor_tensor(
                        out=gpl[:, i:i + 1], in0=lg[i], scalar=sol0,
                        in1=lc[i], op0=ALU.mult, op1=ALU.add,
                    )
                    nc.vector.scalar_tensor_tensor(
                        out=gpr[:, i:i + 1], in0=rg[i], scalar=sol1,
                        in1=rc[i], op0=ALU.mult, op1=ALU.add,
                    )
                    nc.vector.tensor_copy(out=ge[:, i, 0:1], in_=gpl[:, i:i + 1])
                    nc.vector.tensor_copy(out=ge[:, i, 1:2], in_=gpr[:, i:i + 1])
                    o = g.add(gpl[:, i:i + 1], gpr[:, i:i + 1])
                    nc.vector.tensor_scalar_mul(out=ge[:, i, 2:3], in0=o, scalar1=0.5)
                dff = geop.tile([B, 3], F32)
                nc.vector.tensor_tensor(out=dff, in0=gpl, in1=gpr, op=ALU.subtract)
                nc.vector.tensor_tensor(out=dff, in0=dff, in1=dff, op=ALU.mult)
                d2 = g.t()
                nc.vector.tensor_reduce(out=d2, in_=dff, axis=AX.X, op=ALU.add)
                dist = g.t()
                nc.scalar.activation(out=dist, in_=d2, func=ACTF.Sqrt)
                for i in range(3):
                    nc.scalar.copy(out=ge[:, i, 7:8], in_=dist)

                nc.scalar.dma_start(out=out_p[:, :, 2 * SL + 71:2 * SL + 79], in_=ge)
    _legalize_waits(nc)
    return nc


def _prep(inputs):
    f32 = np.float32
    x = np.ascontiguousarray(inputs["x"].reshape(B, DIN), dtype=f32)
    W = np.asarray(inputs["enc_W"], dtype=f32)
    Wp = np.concatenate([W[:, :400], W[:, 545:556]], axis=1)  # [DIN, 411]
    enc_b = np.asarray(inputs["enc_b"], dtype=f32)
    bp = np.concatenate([enc_b[:400], enc_b[545:556]])
    bvec = np.concatenate(
        [bp / NCORES, np.array([1.0 / NCORES], f32)]
    ).reshape(1, NCOLS + 1).astype(f32)
    tmpl = np.asarray(inputs["v_template"], dtype=f32)  # [V, 3]
    basis = np.asarray(inputs["shape_basis"], dtype=f32)  # [400, V, 3]
    cam = np.ascontiguousarray(
        np.asarray(inputs["camera_parameters"], dtype=f32).reshape(B, 12)
    )
    lm = np.asarray(inputs["landmarks"])
    mlm = np.asarray(inputs["masked_landmarks"])
    fmask = np.asarray(inputs["face_mask"])
    lmask = np.asarray(inputs["left_eyeball_mask"])
    rmask = np.asarray(inputs["right_eyeball_mask"])
    fl_idx = fmask[mlm]  # verts behind the 68 output landmarks
    idx4 = lm[np.array([19, 22, 25, 28])]
    idx2 = lm[np.array([14, 18])]

    # synthetic extra columns [400, 72, 3] / [72, 3]
    ex_b = np.concatenate([
        basis[:, fl_idx, :],
        basis[:, lmask, :].mean(axis=1, keepdims=True),
        basis[:, rmask, :].mean(axis=1, keepdims=True),
        (basis[:, idx4, :].mean(axis=1, keepdims=True)
         + basis[:, idx2, :].mean(axis=1, keepdims=True)) / 2.0,
        basis.mean(axis=1, keepdims=True),
    ], axis=1)
    ex_t = np.concatenate([
        tmpl[fl_idx],
        tmpl[lmask].mean(axis=0, keepdims=True),
        tmpl[rmask].mean(axis=0, keepdims=True),
        (tmpl[idx4].mean(axis=0, keepdims=True)
         + tmpl[idx2].mean(axis=0, keepdims=True)) / 2.0,
        tmpl.mean(axis=0, keepdims=True),
    ], axis=0)

    eye = np.eye(B, dtype=f32)
    in_maps = []
    for c in range(NCORES):
        k0 = c * KSH
        xs = x[:, k0:k0 + KSH].T  # [KSH, B] f32
        xh = xs.astype(BF)
        xl = (xs - xh.astype(f32)).astype(BF)
        xw = np.ascontiguousarray(
            np.stack([
                xh.reshape(KT, 128, B).transpose(1, 0, 2),
                xl.reshape(KT, 128, B).transpose(1, 0, 2),
            ], axis=2)
        )  # [128, KT, 2, B] bf16
        ws = Wp[k0:k0 + KSH]  # [KSH, 411] f32
        wh = ws.astype(BF)
        wl = (ws - wh.astype(f32)).astype(BF)
        wch = np.ascontiguousarray(
            np.stack([
                wh.reshape(NCH, TPC, 128, NCOLS).transpose(0, 2, 1, 3),
                wl.reshape(NCH, TPC, 128, NCOLS).transpose(0, 2, 1, 3),
            ], axis=3)
        )  # [NCH, 128, TPC, 2, 411] bf16

        lo = c * SL
        verts = fmask[lo:min(lo + SL, VM)]
        nsl = len(verts)
        blk = np.zeros((400, N2), f32)
        trow = np.zeros(N2, f32)
        for l in range(3):
            blk[:, l * PL:l * PL + nsl] = basis[:, verts, l]
            blk[:, l * PL + SL:l * PL + SL + 72] = ex_b[:, :, l]
            trow[l * PL:l * PL + nsl] = tmpl[verts, l]
            trow[l * PL + SL:l * PL + SL + 72] = ex_t[:, l]
        bh = np.zeros((128, 4, N2), f32)
        for kt in range(3):
            bh[:, kt, :] = blk[kt * 128:(kt + 1) * 128]
        bh[0:16, 3, :] = blk[384:400]
        bh[27, 3, :] = trow  # coefficient = exact 1.0 from AR col 411
        in_maps.append({
            "xw": xw,
            "wch": wch,
            "bvec": bvec,
            "basis": np.ascontiguousarray(bh),
            "cam": cam,
            "eye": eye,
        })
    return in_maps


def _run(inputs, trace=False):
    in_maps = _prep(inputs)
    nc = build_graph()
    res = run_bass_kernel_spmd(
        nc, in_maps, core_ids=list(range(NCORES)), trace=trace
    )
    full = np.empty((B, 3, NOUT), np.float32)
    for c in range(NCORES):
        r = res.results[c]["out"]  # [B, 3, 975]
        lo = c * SL
        w = min(SL, VM - lo)
        full[:, :, lo:lo + w] = r[:, :, 0:w]
        full[:, :, VM + lo:VM + lo + w] = r[:, :, SL:SL + w]
    r0 = res.results[0]["out"]
    full[:, :, 2 * VM:NOUT] = r0[:, :, 2 * SL:2 * SL + 79]
    return np.ascontiguousarray(full.transpose(0, 2, 1)), res


def kernel(**inputs):
    out, _ = _run(inputs, trace=False)
    return out
